# revision 58
# baseline (speedup 1.0000x reference)
"""Trainium2 Bass kernel for nn_EnhancedGraphEncoder (RGCN x2 + pooled attention head).

Self-contained: builds and runs an 8-core SPMD Bass kernel.
Sharding: nodes split into 8 equal contiguous ranges (6250 each); edges assigned
to the core owning their dst node. Weights replicated. h1 exchanged between the
two RGCN layers with an on-device AllGather (bf16).

Key algebraic restructurings (exact math, different association):
  - h0 = x@W_init + b_init is never materialized: layer-1 msg/root weights are
    fused:  A1_r = W_init @ W_rel1[r],  A1_root = W_init @ W_root1,
    bias rows c1_r = b_init @ W_rel1[r] folded in via a per-node
    relation-presence matmul (sum_e w_e c_{r_e} = sum_{r present at dst} c_r,
    since per-(rel,dst) mean weights w_e sum to 1).
  - per-edge mean normalization w_e = 1/cnt_{rel,dst} precomputed on host
    (pure graph structure), applied as per-partition scale on messages.
  - the edge-attention head collapses to per-(graph,relation) counts:
    rel_e rows take only 8 distinct values.
"""

import sys
import numpy as np

sys.path.insert(0, "/opt/trn_rl_repo")

import concourse.bass as bass
import concourse.bacc as bacc
import concourse.tile as tile
import concourse.mybir as mybir
from concourse.bass_utils import run_bass_kernel_spmd
import ml_dtypes

bf16 = ml_dtypes.bfloat16

N, E, R, H, L, OUT, G = 50000, 100000, 8, 256, 768, 768, 64
C = 8            # cores
P = 128
NSH = N // C     # 6250 nodes per core
NB = (NSH + P - 1) // P   # 49 blocks per core

F32 = mybir.dt.float32
BF16 = mybir.dt.bfloat16
I32 = mybir.dt.int32

AF = mybir.ActivationFunctionType
ALU = mybir.AluOpType


# ---------------------------------------------------------------- host prep

def _part_major(arr, ntiles, pad_val):
    """[ntiles*128] -> [128, ntiles] partition-major; pads with pad_val."""
    out = np.full(ntiles * P, pad_val, dtype=arr.dtype)
    out[: len(arr)] = arr
    return np.ascontiguousarray(out.reshape(ntiles, P).T)


def host_prep(x, edge_index, edge_attr, ptr):
    src = np.asarray(edge_index[0], dtype=np.int64)
    dst = np.asarray(edge_index[1], dtype=np.int64)
    ea = np.asarray(edge_attr, dtype=np.int64)
    ptr = np.asarray(ptr, dtype=np.int64)
    node_batch = (np.searchsorted(ptr, np.arange(N), side="right") - 1).astype(np.int64)

    # per (rel, dst) in-edge counts -> mean weights + presence
    cnt = np.zeros((R, N), dtype=np.float64)
    np.add.at(cnt, (ea, dst), 1.0)
    w_edge = (1.0 / cnt[ea, dst]).astype(np.float32)
    pres = (cnt > 0).astype(np.float32)          # [R, N]
    deg = cnt.sum(axis=0).astype(np.int64)       # in-degree per node

    # node -> (core, block, slot) rebalancing: equalize per-(core,block)
    # in-edge counts; outputs are reassembled on host, so any permutation works.
    import heapq
    ncap = np.full((C, NB), P, dtype=np.int64)
    ncap[:, NB - 1] = NSH - (NB - 1) * P
    heap = [(0, int(c), int(b)) for c in range(C) for b in range(NB)]
    heapq.heapify(heap)
    used = np.zeros((C, NB), dtype=np.int64)
    node_core = np.empty(N, dtype=np.int64)
    node_loc = np.empty(N, dtype=np.int64)
    for n in np.argsort(-deg, kind="stable"):
        while True:
            w_, c, b = heapq.heappop(heap)
            if used[c, b] < ncap[c, b]:
                break
        node_core[n] = c
        node_loc[n] = b * P + used[c, b]
        used[c, b] += 1
        if used[c, b] < ncap[c, b]:
            heapq.heappush(heap, (w_ + int(deg[n]), c, b))
    glob_of = (node_core * NSH + node_loc).astype(np.int64)  # h1full row
    core_of = node_core[dst]
    eloc = node_loc[dst]

    # ---- pass-1 schedule: relation-major buckets, padded to 128 per rel,
    # tile counts uniform across cores (max over cores per rel).
    rel_lists = []
    for c in range(C):
        eidx = np.nonzero(core_of == c)[0]
        d_loc = eloc[eidx]
        o1 = np.lexsort((d_loc, ea[eidx]))
        e1 = eidx[o1]
        rel_lists.append([e1[ea[e1] == r] for r in range(R)])
    T1_r = [max(int(np.ceil(len(rel_lists[c][r]) / P)) for c in range(C)) for r in range(R)]
    T1 = sum(T1_r)
    E1p = T1 * P

    src1 = np.zeros((C, E1p), dtype=np.int32)
    src1g = np.zeros((C, E1p), dtype=np.int32)
    w1 = np.zeros((C, E1p), dtype=np.float32)
    pos1 = [dict() for _ in range(C)]
    for c in range(C):
        off = 0
        for r in range(R):
            el = rel_lists[c][r]
            src1[c, off:off + len(el)] = src[el]
            src1g[c, off:off + len(el)] = glob_of[src[el]]
            w1[c, off:off + len(el)] = w_edge[el]
            for k, e in enumerate(el):
                pos1[c][e] = off + k
            off += T1_r[r] * P

    # ---- pass-2 schedule: dst-block-major, per-block tiles uniform across cores
    blk_lists = []
    for c in range(C):
        eidx = np.nonzero(core_of == c)[0]
        d_loc = eloc[eidx]
        o2 = np.argsort(d_loc, kind="stable")
        e2 = eidx[o2]
        blk = (d_loc[o2]) // P
        blk_lists.append([e2[blk == b] for b in range(NB)])
    T2_b = [max(int(np.ceil(len(blk_lists[c][b]) / P)) for c in range(C)) for b in range(NB)]
    T2 = sum(T2_b)
    E2p = max(T2, 1) * P

    gidx2 = np.zeros((C, E2p), dtype=np.int32)
    slot2 = np.full((C, E2p), -1.0, dtype=np.float32)
    for c in range(C):
        off = 0
        for b in range(NB):
            el = blk_lists[c][b]
            d_loc = eloc[el]
            gidx2[c, off:off + len(el)] = [pos1[c][e] for e in el]
            slot2[c, off:off + len(el)] = (d_loc % P).astype(np.float32)
            off += T2_b[b] * P

    sched = dict(T1_r=T1_r, T1=T1, T2_b=T2_b, T2=T2, E1p=E1p, E2p=E2p)
    # ---- per-node relation presence (9 x NSH, row0 = ones) per core, loc order
    paug = np.zeros((C, R + 1, NSH), dtype=np.float32)
    for c in range(C):
        paug[c, 0, :] = 1.0
        sel = node_core == c
        paug[c, 1:, node_loc[sel]] = pres[:, sel].T

    # ---- pooling / head constants (loc order)
    batchslot = np.zeros((C, P, NB), dtype=np.float32)
    for c in range(C):
        sel = node_core == c
        nb_loc = np.empty(NSH, dtype=np.float32)
        nb_loc[node_loc[sel]] = node_batch[sel].astype(np.float32)
        batchslot[c] = _part_major(nb_loc, NB, -1.0)

    cnt_gr = np.zeros((G, R), dtype=np.float64)
    np.add.at(cnt_gr, (node_batch[src], ea), 1.0)
    cnt_gr = cnt_gr.astype(np.float32)
    presneg = (-60.0 * (cnt_gr == 0)).astype(np.float32)
    nvec = cnt_gr.sum(axis=0, keepdims=True).astype(np.float32)   # [1, R]
    gsizes = np.diff(ptr).astype(np.float32)
    cnts_nodes = np.maximum(gsizes, 1.0).reshape(G, 1)

    arrays = dict(
        src1_raw=[src1[c].copy() for c in range(C)],
        node_core=node_core, node_loc=node_loc,
        src1=[_part_major(src1g[c], T1, 0) for c in range(C)],
        w1=[_part_major(w1[c], T1, 0.0) for c in range(C)],
        gidx2=[_part_major(gidx2[c], max(T2, 1), 0) for c in range(C)],
        slot2=[_part_major(slot2[c], max(T2, 1), -1.0) for c in range(C)],
        paug=[np.ascontiguousarray(paug[c]) for c in range(C)],
        batchslot=[np.ascontiguousarray(batchslot[c]) for c in range(C)],
        cnt_gr=cnt_gr, presneg=presneg, nvec=nvec, cnts_nodes=cnts_nodes,
    )
    return sched, arrays


# ---------------------------------------------------------------- builder

def build(sched, ba2_val):
    T1_r, T2_b = sched["T1_r"], sched["T2_b"]
    T1, T2 = sched["T1"], sched["T2"]
    E1p, E2p = sched["E1p"], sched["E2p"]

    nc = bacc.Bacc("TRN2", target_bir_lowering=False, debug=False, num_devices=C)

    def din(name, shape, dt=F32):
        return nc.dram_tensor(name, list(shape), dt, kind="ExternalInput").ap()

    # graph-structure / schedule inputs (per-core content)
    xg1T = din("xg1T", [H, E1p], BF16)
    xTs = din("xTs", [H, NSH], BF16)
    src1 = din("src1", [P, T1], I32)
    w1 = din("w1", [P, T1])
    gidx2 = din("gidx2", [P, max(T2, 1)], I32)
    slot2 = din("slot2", [P, max(T2, 1)])
    paug = din("paug", [R + 1, NSH])
    batchslot = din("batchslot", [P, NB])
    iota128 = din("iota128", [P, P])
    iota64 = din("iota64", [P, G])
    ident_bf = din("ident_bf", [P, P], BF16)
    ident_f32 = din("ident_f32", [P, P])
    ones1x64 = din("ones1x64", [1, G])
    cnt_gr = din("cnt_gr", [G, R])
    presneg = din("presneg", [G, R])
    nvec = din("nvec", [1, R])
    cnts_nodes = din("cnts_nodes", [G, 1])
    # weights
    KH = H // P   # 2 chunks over 256
    KL = L // P   # 6 chunks over 768
    KO = OUT // P  # 6
    W_init_d = din("W_init", [H, L])
    W_initT_d = din("W_initT", [L, H])
    b_init_col = din("b_init_col", [P, KL])
    b_init_rep8 = din("b_init_rep8", [R, L])
    W_root1_d = din("W_root1", [L, H])
    b1row = din("b1row", [1, H])
    W_rel1_d = din("W_rel1", [R, L, H])
    W_root2_d = din("W_root2", [H, H])
    b2row = din("b2row", [1, H])
    W_rel2_d = din("W_rel2", [R, H, H])
    Wg_d = din("Wg", [H, L])
    bg_col = din("bg_col", [P, KL])
    Wa1_d = din("Wa1", [L, L])
    ba1_col = din("ba1_col", [P, KL])
    Wa2_col = din("Wa2_col", [P, KL])
    rel_tableT = din("rel_tableT", [H, R])
    Wo_d = din("Wo", [2 * L, OUT])
    bo_col = din("bo_col", [P, KO])
    lng_rep = din("lng_rep", [G, OUT])
    lnb_rep = din("lnb_rep", [G, OUT])

    out_nodes = nc.dram_tensor("out_nodes", [NSH, H], F32, kind="ExternalOutput").ap()
    out_graph = nc.dram_tensor("out_graph", [G, OUT], F32, kind="ExternalOutput").ap()

    with tile.TileContext(nc) as tc:
        with (
            tc.tile_pool(name="const", bufs=1) as cp,
            tc.tile_pool(name="dram", bufs=1, space="DRAM") as dr,
        ):
            # ---------------- resident schedule arrays / constants
            src1_sb = cp.tile([P, T1], I32)
            nc.sync.dma_start(out=src1_sb[:], in_=src1[:])
            w1_sb = cp.tile([P, T1], F32)
            nc.sync.dma_start(out=w1_sb[:], in_=w1[:])
            gidx2_sb = cp.tile([P, max(T2, 1)], I32)
            nc.sync.dma_start(out=gidx2_sb[:], in_=gidx2[:])
            slot2_sb = cp.tile([P, max(T2, 1)], F32)
            nc.sync.dma_start(out=slot2_sb[:], in_=slot2[:])
            paug_sb = cp.tile([R + 1, NSH], F32)
            nc.sync.dma_start(out=paug_sb[:], in_=paug[:])
            bslot_sb = cp.tile([P, NB], F32)
            nc.sync.dma_start(out=bslot_sb[:], in_=batchslot[:])
            iota128_sb = cp.tile([P, P], F32)
            nc.sync.dma_start(out=iota128_sb[:], in_=iota128[:])
            iota64_sb = cp.tile([P, G], F32)
            nc.sync.dma_start(out=iota64_sb[:], in_=iota64[:])
            identbf_sb = cp.tile([P, P], BF16)
            nc.sync.dma_start(out=identbf_sb[:], in_=ident_bf[:])
            identf_sb = cp.tile([P, P], F32)
            nc.sync.dma_start(out=identf_sb[:], in_=ident_f32[:])

            xTs_sb = [cp.tile([P, NSH], BF16, name=f"xTs{k}") for k in range(KH)]
            for k in range(KH):
                nc.sync.dma_start(out=xTs_sb[k][:], in_=xTs[k * P:(k + 1) * P, :])
            h1T_sb = [cp.tile([P, NSH], BF16, name=f"h1T{k}") for k in range(KH)]

            # fused weights (resident, bf16)
            A1 = [[cp.tile([P, H], BF16, name=f"A1_{r}_{k}") for k in range(KH)] for r in range(R)]
            A1root = [cp.tile([P, H], BF16, name=f"A1root_{k}") for k in range(KH)]
            A2 = [[cp.tile([P, H], BF16, name=f"A2_{r}_{k}") for k in range(KH)] for r in range(R)]
            A2root = [cp.tile([P, H], BF16, name=f"A2root_{k}") for k in range(KH)]
            Caug1_sb = cp.tile([R + 1, H], F32)
            b2_sb = cp.tile([1, H], F32)
            nc.sync.dma_start(out=b2_sb[:], in_=b2row[:])

            # big DRAM intermediates
            caug_dram = dr.tile([R + 1, H], F32)
            msg1 = dr.tile([E1p, H], BF16)
            msg2 = dr.tile([E1p, H], BF16)
            h1f32 = dr.tile([NSH, H], F32)
            h1bf_sh = dr.tile([NSH, H], BF16)
            h1full = dr.tile([N, H], BF16, addr_space="Shared")
            poolin = dr.tile([G, H], F32)
            poolout = dr.tile([G, H], F32, addr_space="Shared")

            # ---------------- phase W: fused weight prep
            with (
                tc.tile_pool(name="wp", bufs=2) as wp,
                tc.tile_pool(name="wpp", bufs=2, space="PSUM") as wpp,
            ):
                WiT = [wp.tile([P, H], F32, name=f"WiT{k}", bufs=1) for k in range(KL)]
                for k in range(KL):
                    nc.sync.dma_start(out=WiT[k][:], in_=W_initT_d[k * P:(k + 1) * P, :])
                binit_sb = wp.tile([P, KL], F32, bufs=1)
                nc.sync.dma_start(out=binit_sb[:], in_=b_init_col[:])

                def fuse(dst_tiles, rhs_dram, bias_row_idx):
                    """dst[mc] = (W_init @ rhs)[mc*128:(mc+1)*128, :] as bf16;
                    also Caug1[bias_row_idx] = b_init @ rhs (f32)."""
                    rhs_t = [wp.tile([P, H], F32, name="fuserhs", tag="fuserhs", bufs=3)
                             for _ in range(KL)]
                    for k in range(KL):
                        nc.sync.dma_start(out=rhs_t[k][:], in_=rhs_dram[k * P:(k + 1) * P, :])
                    for mc in range(KH):
                        ps = wpp.tile([P, H], F32, name="fusepsum", tag="fps")
                        for k in range(KL):
                            nc.tensor.matmul(
                                out=ps[:], lhsT=WiT[k][:, mc * P:(mc + 1) * P],
                                rhs=rhs_t[k][:], start=(k == 0), stop=(k == KL - 1))
                        nc.vector.tensor_copy(out=dst_tiles[mc][:], in_=ps[:])
                    psb = wpp.tile([1, H], F32, name="fusebias", tag="fpb")
                    for k in range(KL):
                        nc.tensor.matmul(out=psb[:], lhsT=binit_sb[:, k:k + 1],
                                         rhs=rhs_t[k][:], start=(k == 0), stop=(k == KL - 1))
                    stage = wp.tile([1, H], F32, name="cstage", tag="cstage", bufs=2)
                    nc.vector.tensor_copy(out=stage[:], in_=psb[:])
                    nc.sync.dma_start(
                        out=caug_dram[bias_row_idx:bias_row_idx + 1, :], in_=stage[:])

                fuse(A1root, W_root1_d, 0)
                for r in range(R):
                    fuse(A1[r], W_rel1_d[r], 1 + r)
                nc.sync.dma_start(out=Caug1_sb[:], in_=caug_dram[:])
                # Caug1 row0 += b1
                b1_sb = wp.tile([1, H], F32)
                nc.sync.dma_start(out=b1_sb[:], in_=b1row[:])
                nc.vector.tensor_add(out=Caug1_sb[0:1, :], in0=Caug1_sb[0:1, :], in1=b1_sb[:])

                # layer-2 weights: straight bf16 casts
                for r in range(R):
                    for k in range(KH):
                        t = wp.tile([P, H], F32, name="w2ld", tag="w2ld", bufs=3)
                        nc.sync.dma_start(out=t[:], in_=W_rel2_d[r][k * P:(k + 1) * P, :])
                        nc.vector.tensor_copy(out=A2[r][k][:], in_=t[:])
                for k in range(KH):
                    t = wp.tile([P, H], F32, name="w2r", tag="w2ld", bufs=3)
                    nc.sync.dma_start(out=t[:], in_=W_root2_d[k * P:(k + 1) * P, :])
                    nc.vector.tensor_copy(out=A2root[k][:], in_=t[:])

            # ---------------- message pass (shared for both layers)
            def pass1(src_dram_bf, Ar, msgbuf):
                with (
                    tc.tile_pool(name="p1", bufs=6) as sp,
                    tc.tile_pool(name="p1p", bufs=4, space="PSUM") as pp,
                ):
                    t = 0
                    for r in range(R):
                        for _ in range(T1_r[r]):
                            g = sp.tile([P, H], BF16, name="g", tag="g")
                            nc.gpsimd.indirect_dma_start(
                                out=g[:], out_offset=None, in_=src_dram_bf[:],
                                in_offset=bass.IndirectOffsetOnAxis(
                                    ap=src1_sb[:, t:t + 1], axis=0))
                            gT = []
                            for k in range(KH):
                                tp = pp.tile([P, P], BF16, name="tp", tag="tp")
                                nc.tensor.transpose(
                                    out=tp[:], in_=g[:, k * P:(k + 1) * P],
                                    identity=identbf_sb[:])
                                gk = sp.tile([P, P], BF16, name="gT", tag=f"gT{k}")
                                nc.vector.tensor_copy(out=gk[:], in_=tp[:])
                                gT.append(gk)
                            mm = pp.tile([P, H], F32, name="mm", tag="mm")
                            for k in range(KH):
                                nc.tensor.matmul(out=mm[:], lhsT=gT[k][:], rhs=Ar[r][k][:],
                                                 start=(k == 0), stop=(k == KH - 1))
                            mb = sp.tile([P, H], BF16, name="mb", tag="mb")
                            nc.scalar.activation(out=mb[:], in_=mm[:], func=AF.Copy,
                                                 scale=w1_sb[:, t:t + 1])
                            nc.sync.dma_start(out=msgbuf[t * P:(t + 1) * P, :], in_=mb[:])
                            t += 1

            def pass2(msgbuf, Aroot, lhsT_sb, caug_k, on_block):
                with (
                    tc.tile_pool(name="p2", bufs=6) as sp,
                    tc.tile_pool(name="p2p", bufs=4, space="PSUM") as pp,
                ):
                    t = 0
                    for b in range(NB):
                        nb_sz = min(P, NSH - b * P)
                        acc = pp.tile([P, H], F32, name="acc", tag="acc")
                        for k in range(KH):
                            nc.tensor.matmul(
                                out=acc[:nb_sz], lhsT=lhsT_sb[k][:, b * P:b * P + nb_sz],
                                rhs=Aroot[k][:], start=(k == 0), stop=False)
                        nc.tensor.matmul(
                            out=acc[:nb_sz], lhsT=paug_sb[:caug_k, b * P:b * P + nb_sz],
                            rhs=(Caug1_sb[:caug_k, :] if caug_k > 1 else b2_sb[:]),
                            start=False, stop=(T2_b[b] == 0))
                        for _ in range(T2_b[b]):
                            mg = sp.tile([P, H], BF16, name="mg", tag="mg")
                            nc.gpsimd.indirect_dma_start(
                                out=mg[:], out_offset=None, in_=msgbuf[:],
                                in_offset=bass.IndirectOffsetOnAxis(
                                    ap=gidx2_sb[:, t:t + 1], axis=0))
                            oh = sp.tile([P, P], BF16, name="oh", tag="oh")
                            nc.vector.tensor_tensor(
                                out=oh[:], in0=slot2_sb[:, t:t + 1].to_broadcast([P, P]),
                                in1=iota128_sb[:], op=ALU.is_equal)
                            last = t == sum(T2_b[:b + 1]) - 1
                            nc.tensor.matmul(out=acc[:nb_sz], lhsT=oh[:, :nb_sz], rhs=mg[:],
                                             start=False, stop=last)
                            t += 1
                        on_block(b, nb_sz, acc, sp, pp)

            # ---------------- layer 1: host-pregathered feature-major stage;
            # 64-padded runs write msg rows directly at partition offsets.
            def pass1_direct(Ar, msgbuf):
                GB = 16
                rels = [r for r in range(R) for _ in range(T1_r[r])]
                with (
                    tc.tile_pool(name="p1d", bufs=6) as sp,
                    tc.tile_pool(name="p1dp", bufs=6, space="PSUM") as pp,
                ):
                    for g0 in range(0, T1, GB):
                        k_gr = min(GB, T1 - g0)
                        gw = [sp.tile([P, GB * P], BF16, name=f"gw{k}",
                                      tag=f"gw{k}") for k in range(KH)]
                        for k in range(KH):
                            nc.sync.dma_start(
                                out=gw[k][:, :k_gr * P],
                                in_=xg1T[k * P:(k + 1) * P, g0 * P:(g0 + k_gr) * P])
                        for j in range(k_gr):
                            t = g0 + j
                            mm = pp.tile([P, H], F32, name="mmd", tag="mmd")
                            for k in range(KH):
                                nc.tensor.matmul(
                                    out=mm[:], lhsT=gw[k][:, j * P:(j + 1) * P],
                                    rhs=Ar[rels[t]][k][:],
                                    start=(k == 0), stop=(k == KH - 1))
                            mb = sp.tile([P, H], BF16, name="mbd", tag="mbd")
                            nc.vector.tensor_scalar_mul(mb[:], mm[:], w1_sb[:, t:t + 1])
                            nc.sync.dma_start(out=msgbuf[t * P:(t + 1) * P, :],
                                              in_=mb[:])

            def l1_block(b, nb_sz, acc, sp, pp):
                h1b = sp.tile([P, H], F32, name="h1b", tag="h1b")
                nc.scalar.activation(out=h1b[:nb_sz], in_=acc[:nb_sz], func=AF.Relu)
                hbf = sp.tile([P, H], BF16, name="hbf", tag="hbf")
                nc.vector.tensor_copy(out=hbf[:nb_sz], in_=h1b[:nb_sz])
                nc.sync.dma_start(out=h1f32[b * P:b * P + nb_sz, :], in_=h1b[:nb_sz])
                nc.sync.dma_start(out=h1bf_sh[b * P:b * P + nb_sz, :], in_=hbf[:nb_sz])
                for k in range(KH):
                    tp = pp.tile([P, P], BF16, name="tph", tag="tph")
                    nc.tensor.transpose(out=tp[:, :nb_sz], in_=hbf[:nb_sz, k * P:(k + 1) * P],
                                        identity=identbf_sb[:nb_sz, :nb_sz])
                    nc.vector.tensor_copy(out=h1T_sb[k][:, b * P:b * P + nb_sz],
                                          in_=tp[:, :nb_sz])

            # ---------------- early head (pooling-independent attention branch)
            rcT = [cp.tile([P, G], F32, name=f"rcT{m}") for m in range(KL)]
            with (
                tc.tile_pool(name="eh", bufs=2) as hp,
                tc.tile_pool(name="ehp", bufs=2, space="PSUM") as hpp,
            ):
                def hps(p_, f_):
                    return hpp.tile([p_, f_], F32, name="ehps", tag="ehps")

                # rel8 [R, L] = rel_table @ W_init + b_init
                rtT = [hp.tile([P, R], F32, name=f"rtT{k}", bufs=1) for k in range(KH)]
                for k in range(KH):
                    nc.sync.dma_start(out=rtT[k][:], in_=rel_tableT[k * P:(k + 1) * P, :])
                Wi_sb = [hp.tile([P, L], F32, name=f"Wi{k}", bufs=1) for k in range(KH)]
                for k in range(KH):
                    nc.sync.dma_start(out=Wi_sb[k][:], in_=W_init_d[k * P:(k + 1) * P, :])
                rel8 = hp.tile([R, L], F32, bufs=1)
                for half in range(2):
                    sl = slice(half * (L // 2), (half + 1) * (L // 2))
                    ps = hps(R, L // 2)
                    for k in range(KH):
                        nc.tensor.matmul(out=ps[:], lhsT=rtT[k][:],
                                         rhs=Wi_sb[k][:, sl], start=(k == 0), stop=(k == KH - 1))
                    nc.vector.tensor_copy(out=rel8[:, sl], in_=ps[:])
                bi8 = hp.tile([R, L], F32, bufs=1)
                nc.sync.dma_start(out=bi8[:], in_=b_init_rep8[:])
                nc.vector.tensor_add(out=rel8[:], in0=rel8[:], in1=bi8[:])
                # rel8^T [L, R]
                r8T = [hp.tile([P, R], F32, name=f"r8T{k}", bufs=1) for k in range(KL)]
                for k in range(KL):
                    tp = hps(P, R)
                    nc.tensor.transpose(out=tp[:, :R], in_=rel8[:, k * P:(k + 1) * P],
                                        identity=identf_sb[:R, :R])
                    nc.vector.tensor_copy(out=r8T[k][:], in_=tp[:, :R])
                # t1T = tanh(Wa1^T @ rel8^T + ba1) [L, R]
                Wa1_sb = [hp.tile([P, L], F32, name=f"Wa1{k}", bufs=1) for k in range(KL)]
                for k in range(KL):
                    nc.sync.dma_start(out=Wa1_sb[k][:], in_=Wa1_d[k * P:(k + 1) * P, :])
                ba1_sb = hp.tile([P, KL], F32, bufs=1)
                nc.sync.dma_start(out=ba1_sb[:], in_=ba1_col[:])
                t1T = [hp.tile([P, R], F32, name=f"t1T{m}", bufs=1) for m in range(KL)]
                for m in range(KL):
                    ps = hps(P, R)
                    for k in range(KL):
                        nc.tensor.matmul(out=ps[:], lhsT=Wa1_sb[k][:, m * P:(m + 1) * P],
                                         rhs=r8T[k][:], start=(k == 0), stop=(k == KL - 1))
                    nc.scalar.activation(out=t1T[m][:], in_=ps[:], func=AF.Tanh,
                                         bias=ba1_sb[:, m:m + 1])
                # s8 [1, R]
                Wa2_sb = hp.tile([P, KL], F32, bufs=1)
                nc.sync.dma_start(out=Wa2_sb[:], in_=Wa2_col[:])
                ps8 = hps(1, R)
                for k in range(KL):
                    nc.tensor.matmul(out=ps8[:], lhsT=Wa2_sb[:, k:k + 1],
                                     rhs=t1T[k][:], start=(k == 0), stop=(k == KL - 1))
                s8 = hp.tile([1, R], F32, bufs=1)
                nc.scalar.add(out=s8[:], in_=ps8[:], add=float(ba2_val))
                # global softmax with counts
                smax = hp.tile([1, 1], F32, bufs=1)
                nc.vector.tensor_reduce(out=smax[:], in_=s8[:], axis=mybir.AxisListType.X,
                                        op=ALU.max)
                nsmax = hp.tile([1, 1], F32, bufs=1)
                nc.vector.tensor_scalar_mul(nsmax[:], smax[:], -1.0)
                e8 = hp.tile([1, R], F32, bufs=1)
                nc.scalar.activation(out=e8[:], in_=s8[:], func=AF.Exp, bias=nsmax[:, :1])
                nv_sb = hp.tile([1, R], F32, bufs=1)
                nc.sync.dma_start(out=nv_sb[:], in_=nvec[:])
                wsum = hp.tile([1, R], F32, bufs=1)
                nc.vector.tensor_mul(out=wsum[:], in0=e8[:], in1=nv_sb[:])
                den = hp.tile([1, 1], F32, bufs=1)
                nc.vector.tensor_reduce(out=den[:], in_=wsum[:], axis=mybir.AxisListType.X,
                                        op=ALU.add)
                rden = hp.tile([1, 1], F32, bufs=1)
                nc.vector.reciprocal(out=rden[:], in_=den[:])
                aval = hp.tile([1, R], F32, bufs=1)
                nc.vector.tensor_scalar_mul(aval[:], e8[:], rden[:, :1])
                # replicate aval over 64 partitions
                ones64_sb = hp.tile([1, G], F32, bufs=1)
                nc.sync.dma_start(out=ones64_sb[:], in_=ones1x64[:])
                avps = hps(G, R)
                nc.tensor.matmul(out=avps[:], lhsT=ones64_sb[:], rhs=aval[:],
                                 start=True, stop=True)
                avrep = hp.tile([G, R], F32, bufs=1)
                nc.vector.tensor_copy(out=avrep[:], in_=avps[:])
                # segment softmax via counts
                cg_sb = hp.tile([G, R], F32, bufs=1)
                nc.sync.dma_start(out=cg_sb[:], in_=cnt_gr[:])
                pn_sb = hp.tile([G, R], F32, bufs=1)
                nc.sync.dma_start(out=pn_sb[:], in_=presneg[:])
                gmin = hp.tile([G, R], F32, bufs=1)
                nc.vector.tensor_add(out=gmin[:], in0=avrep[:], in1=pn_sb[:])
                gmax = hp.tile([G, 1], F32, bufs=1)
                nc.vector.tensor_reduce(out=gmax[:], in_=gmin[:], axis=mybir.AxisListType.X,
                                        op=ALU.max)
                ngmax = hp.tile([G, 1], F32, bufs=1)
                nc.vector.tensor_scalar_mul(ngmax[:], gmax[:], -1.0)
                eg = hp.tile([G, R], F32, bufs=1)
                nc.scalar.activation(out=eg[:], in_=avrep[:], func=AF.Exp, bias=ngmax[:, :1])
                wden = hp.tile([G, R], F32, bufs=1)
                nc.vector.tensor_mul(out=wden[:], in0=eg[:], in1=cg_sb[:])
                deng = hp.tile([G, 1], F32, bufs=1)
                nc.vector.tensor_reduce(out=deng[:], in_=wden[:], axis=mybir.AxisListType.X,
                                        op=ALU.add)
                dengc = hp.tile([G, 1], F32, bufs=1)
                nc.vector.tensor_scalar_max(dengc[:], deng[:], 1e-30)
                rdeng = hp.tile([G, 1], F32, bufs=1)
                nc.vector.reciprocal(out=rdeng[:], in_=dengc[:])
                coef = hp.tile([G, R], F32, bufs=1)
                nc.vector.tensor_scalar_mul(coef[:], wden[:], rdeng[:, :1])
                # coef^T [R, G]
                cps = hps(R, G)
                nc.tensor.transpose(out=cps[:R, :], in_=coef[:, :R],
                                    identity=identf_sb[:G, :G])
                coefT = hp.tile([R, G], F32, bufs=1)
                nc.vector.tensor_copy(out=coefT[:], in_=cps[:R, :])
                # rel_ctx^T [L, G] = rel8^T @ coef^T
                for m in range(KL):
                    ps = hps(P, G)
                    nc.tensor.matmul(out=ps[:], lhsT=rel8[:, m * P:(m + 1) * P],
                                     rhs=coefT[:], start=True, stop=True)
                    nc.vector.tensor_copy(out=rcT[m][:], in_=ps[:])

            pass1_direct(A1, msg1)
            pass2(msg1, A1root, xTs_sb, R + 1, l1_block)

            # ---------------- AllGather h1 (bf16)
            nc.gpsimd.collective_compute(
                "AllGather", ALU.bypass,
                ins=[h1bf_sh.opt()], outs=[h1full.opt()],
                replica_groups=[list(range(C))])

            # ---------------- layer 2 (+ residual + pooling partials)
            poolsum_sb = cp.tile([G, H], F32)
            nc.vector.memset(poolsum_sb[:], 0.0)

            def l2_block(b, nb_sz, acc, sp, pp):
                t = sp.tile([P, H], F32, name="t2", tag="t2")
                nc.scalar.activation(out=t[:nb_sz], in_=acc[:nb_sz], func=AF.Relu)
                h1l = sp.tile([P, H], F32, name="h1l", tag="h1l")
                nc.sync.dma_start(out=h1l[:nb_sz], in_=h1f32[b * P:b * P + nb_sz, :])
                h2 = sp.tile([P, H], F32, name="h2", tag="h2")
                if nb_sz < P:
                    nc.vector.memset(h2[:], 0.0)
                nc.vector.tensor_add(out=h2[:nb_sz], in0=t[:nb_sz], in1=h1l[:nb_sz])
                nc.sync.dma_start(out=out_nodes[b * P:b * P + nb_sz, :], in_=h2[:nb_sz])
                oh64 = sp.tile([P, G], F32, name="oh64", tag="oh64")
                nc.vector.tensor_tensor(
                    out=oh64[:], in0=bslot_sb[:, b:b + 1].to_broadcast([P, G]),
                    in1=iota64_sb[:], op=ALU.is_equal)
                pps = pp.tile([G, H], F32, name="pps", tag="pps")
                nc.tensor.matmul(out=pps[:], lhsT=oh64[:], rhs=h2[:], start=True, stop=True)
                nc.vector.tensor_add(out=poolsum_sb[:], in0=poolsum_sb[:], in1=pps[:])

            pass1(h1full, A2, msg2)
            pass2(msg2, A2root, h1T_sb, 1, l2_block)

            # ---------------- AllReduce pooled sums
            nc.sync.dma_start(out=poolin[:], in_=poolsum_sb[:])
            nc.gpsimd.collective_compute(
                "AllReduce", ALU.add,
                ins=[poolin.opt()], outs=[poolout.opt()],
                replica_groups=[list(range(C))])

            # ---------------- head
            with (
                tc.tile_pool(name="hd", bufs=2) as hp,
                tc.tile_pool(name="hdp", bufs=2, space="PSUM") as hpp,
            ):
                K2L = 2 * L // P  # 12

                def hps(p_, f_):
                    return hpp.tile([p_, f_], F32, name="hps", tag="hps")

                # pooled mean [G, H]
                sums = hp.tile([G, H], F32, bufs=1)
                nc.sync.dma_start(out=sums[:], in_=poolout[:])
                cnts_sb = hp.tile([G, 1], F32, bufs=1)
                nc.sync.dma_start(out=cnts_sb[:], in_=cnts_nodes[:])
                rc = hp.tile([G, 1], F32, bufs=1)
                nc.vector.reciprocal(out=rc[:], in_=cnts_sb[:])
                mp = hp.tile([G, H], F32, bufs=1)
                nc.vector.tensor_scalar_mul(mp[:], sums[:], rc[:, :1])
                # meanpool^T [H, G]
                mpT = [hp.tile([P, G], F32, name=f"mpT{k}", bufs=1) for k in range(KH)]
                for k in range(KH):
                    tp = hps(P, G)
                    nc.tensor.transpose(out=tp[:], in_=mp[:, k * P:(k + 1) * P],
                                        identity=identf_sb[:G, :G])
                    nc.vector.tensor_copy(out=mpT[k][:], in_=tp[:])
                # graph_emb^T [L, G] = Wg^T @ mp^T + bg
                Wg_sb = [hp.tile([P, L], F32, name=f"Wg{k}", bufs=1) for k in range(KH)]
                for k in range(KH):
                    nc.sync.dma_start(out=Wg_sb[k][:], in_=Wg_d[k * P:(k + 1) * P, :])
                bg_sb = hp.tile([P, KL], F32, bufs=1)
                nc.sync.dma_start(out=bg_sb[:], in_=bg_col[:])
                geT = [hp.tile([P, G], F32, name=f"geT{m}", bufs=1) for m in range(KL)]
                for m in range(KL):
                    ps = hps(P, G)
                    for k in range(KH):
                        nc.tensor.matmul(out=ps[:], lhsT=Wg_sb[k][:, m * P:(m + 1) * P],
                                         rhs=mpT[k][:], start=(k == 0), stop=(k == KH - 1))
                    nc.vector.tensor_scalar_add(geT[m][:], ps[:], bg_sb[:, m:m + 1])

                # comb^T [OUT, G] = Wo^T @ [geT; rcT] + bo
                Wo_sb = [hp.tile([P, OUT], F32, name=f"Wo{k}", bufs=1) for k in range(K2L)]
                for k in range(K2L):
                    nc.sync.dma_start(out=Wo_sb[k][:], in_=Wo_d[k * P:(k + 1) * P, :])
                bo_sb = hp.tile([P, KO], F32, bufs=1)
                nc.sync.dma_start(out=bo_sb[:], in_=bo_col[:])
                cat = geT + rcT
                combT = [hp.tile([P, G], F32, name=f"combT{m}", bufs=1) for m in range(KO)]
                for m in range(KO):
                    ps = hps(P, G)
                    for k in range(K2L):
                        nc.tensor.matmul(out=ps[:], lhsT=Wo_sb[k][:, m * P:(m + 1) * P],
                                         rhs=cat[k][:], start=(k == 0), stop=(k == K2L - 1))
                    nc.vector.tensor_scalar_add(combT[m][:], ps[:], bo_sb[:, m:m + 1])
                # transpose back to [G, OUT]
                comb = hp.tile([G, OUT], F32, bufs=1)
                for m in range(KO):
                    tp = hps(G, P)
                    nc.tensor.transpose(out=tp[:G, :], in_=combT[m][:, :G],
                                        identity=identf_sb[:])
                    nc.vector.tensor_copy(out=comb[:, m * P:(m + 1) * P], in_=tp[:G, :])
                # layernorm over OUT
                nmu = hp.tile([G, 1], F32, bufs=1)
                nc.vector.tensor_reduce(out=nmu[:], in_=comb[:], axis=mybir.AxisListType.X,
                                        op=ALU.add)
                nc.vector.tensor_scalar_mul(nmu[:], nmu[:], -1.0 / OUT)
                xc = hp.tile([G, OUT], F32, bufs=1)
                nc.vector.tensor_scalar_add(xc[:], comb[:], nmu[:, :1])
                sq = hp.tile([G, OUT], F32, bufs=1)
                vsum = hp.tile([G, 1], F32, bufs=1)
                nc.scalar.activation(out=sq[:], in_=xc[:], func=AF.Square, accum_out=vsum[:])
                var = hp.tile([G, 1], F32, bufs=1)
                nc.vector.tensor_scalar(out=var[:], in0=vsum[:], scalar1=1.0 / OUT,
                                        scalar2=1e-5, op0=ALU.mult, op1=ALU.add)
                sd = hp.tile([G, 1], F32, bufs=1)
                nc.scalar.activation(out=sd[:], in_=var[:], func=AF.Sqrt)
                rsd = hp.tile([G, 1], F32, bufs=1)
                nc.vector.reciprocal(out=rsd[:], in_=sd[:])
                y = hp.tile([G, OUT], F32, bufs=1)
                nc.vector.tensor_scalar_mul(y[:], xc[:], rsd[:, :1])
                lg = hp.tile([G, OUT], F32, bufs=1)
                nc.sync.dma_start(out=lg[:], in_=lng_rep[:])
                lb = hp.tile([G, OUT], F32, bufs=1)
                nc.sync.dma_start(out=lb[:], in_=lnb_rep[:])
                nc.vector.tensor_mul(out=y[:], in0=y[:], in1=lg[:])
                nc.vector.tensor_add(out=y[:], in0=y[:], in1=lb[:])
                nc.sync.dma_start(out=out_graph[:], in_=y[:])

    nc.compile()
    return nc


# ---------------------------------------------------------------- runner

_CACHE = {}


def kernel(x, edge_index, edge_attr, batch, ptr, W_init, b_init, rel_table,
           W_root1, W_rel1, b1, W_root2, W_rel2, b2, Wg, bg, Wa1, ba1, Wa2, ba2,
           Wo, bo, ln_g, ln_b, _run_kwargs=None):
    x = np.asarray(x, dtype=np.float32)
    sched, arr = host_prep(x, np.asarray(edge_index), np.asarray(edge_attr),
                           np.asarray(ptr))

    key = (tuple(sched["T1_r"]), tuple(sched["T2_b"]),
           float(np.asarray(ba2).ravel()[0]))
    if key not in _CACHE:
        _CACHE.clear()
        _CACHE[key] = build(sched, float(np.asarray(ba2).ravel()[0]))
    nc = _CACHE[key]

    f32c = lambda a: np.ascontiguousarray(np.asarray(a, dtype=np.float32))
    iota128 = np.tile(np.arange(P, dtype=np.float32), (P, 1))
    iota64 = np.tile(np.arange(G, dtype=np.float32), (P, 1))
    ident = np.eye(P, dtype=np.float32)

    common = dict(
        iota128=iota128, iota64=iota64,
        ident_bf=ident.astype(bf16), ident_f32=ident,
        ones1x64=np.ones((1, G), np.float32),
        cnt_gr=arr["cnt_gr"], presneg=arr["presneg"], nvec=arr["nvec"],
        cnts_nodes=arr["cnts_nodes"],
        W_init=f32c(W_init), W_initT=f32c(np.asarray(W_init).T),
        b_init_col=np.ascontiguousarray(f32c(b_init).reshape(L // P, P).T),
        b_init_rep8=np.tile(f32c(b_init).reshape(1, L), (R, 1)),
        W_root1=f32c(W_root1), b1row=f32c(b1).reshape(1, H),
        W_rel1=f32c(W_rel1), W_root2=f32c(W_root2), b2row=f32c(b2).reshape(1, H),
        W_rel2=f32c(W_rel2), Wg=f32c(Wg),
        bg_col=np.ascontiguousarray(f32c(bg).reshape(L // P, P).T),
        Wa1=f32c(Wa1),
        ba1_col=np.ascontiguousarray(f32c(ba1).reshape(L // P, P).T),
        Wa2_col=np.ascontiguousarray(f32c(Wa2).reshape(L // P, P).T),
        rel_tableT=f32c(np.asarray(rel_table).T),
        Wo=f32c(Wo),
        bo_col=np.ascontiguousarray(f32c(bo).reshape(OUT // P, P).T),
        lng_rep=np.tile(f32c(ln_g).reshape(1, OUT), (G, 1)),
        lnb_rep=np.tile(f32c(ln_b).reshape(1, OUT), (G, 1)),
    )
    in_maps = []
    node_core, node_loc = arr["node_core"], arr["node_loc"]
    for c in range(C):
        m = dict(common)
        m["xg1T"] = np.ascontiguousarray(x[arr["src1_raw"][c]].T).astype(bf16)
        sel = node_core == c
        xc = np.zeros((NSH, H), dtype=np.float32)
        xc[node_loc[sel]] = x[sel]
        m["xTs"] = np.ascontiguousarray(xc.T).astype(bf16)
        m["src1"] = arr["src1"][c]
        m["w1"] = arr["w1"][c]
        m["gidx2"] = arr["gidx2"][c]
        m["slot2"] = arr["slot2"][c]
        m["paug"] = arr["paug"][c]
        m["batchslot"] = arr["batchslot"][c]
        in_maps.append(m)

    kw = _run_kwargs or {}
    res = run_bass_kernel_spmd(nc, in_maps, core_ids=list(range(C)), **kw)
    kernel._last_result = res
    node_emb = np.empty((N, H), dtype=np.float32)
    for c in range(C):
        sel = node_core == c
        node_emb[sel] = res.results[c]["out_nodes"][node_loc[sel]]
    out_g = res.results[0]["out_graph"]
    return node_emb, out_g


# revision 60
# speedup vs baseline: 1.0180x; 1.0180x over previous
"""Trainium2 Bass kernel for nn_EnhancedGraphEncoder (RGCN x2 + pooled attention head).

Self-contained: builds and runs an 8-core SPMD Bass kernel.
Sharding: nodes split into 8 equal contiguous ranges (6250 each); edges assigned
to the core owning their dst node. Weights replicated. h1 exchanged between the
two RGCN layers with an on-device AllGather (bf16).

Key algebraic restructurings (exact math, different association):
  - h0 = x@W_init + b_init is never materialized: layer-1 msg/root weights are
    fused:  A1_r = W_init @ W_rel1[r],  A1_root = W_init @ W_root1,
    bias rows c1_r = b_init @ W_rel1[r] folded in via a per-node
    relation-presence matmul (sum_e w_e c_{r_e} = sum_{r present at dst} c_r,
    since per-(rel,dst) mean weights w_e sum to 1).
  - per-edge mean normalization w_e = 1/cnt_{rel,dst} precomputed on host
    (pure graph structure), applied as per-partition scale on messages.
  - the edge-attention head collapses to per-(graph,relation) counts:
    rel_e rows take only 8 distinct values.
"""

import sys
import numpy as np

sys.path.insert(0, "/opt/trn_rl_repo")

import concourse.bass as bass
import concourse.bacc as bacc
import concourse.tile as tile
import concourse.mybir as mybir
from concourse.bass_utils import run_bass_kernel_spmd
import ml_dtypes

bf16 = ml_dtypes.bfloat16

N, E, R, H, L, OUT, G = 50000, 100000, 8, 256, 768, 768, 64
C = 8            # cores
P = 128
NSH = N // C     # 6250 nodes per core
NB = (NSH + P - 1) // P   # 49 blocks per core

F32 = mybir.dt.float32
BF16 = mybir.dt.bfloat16
I32 = mybir.dt.int32

AF = mybir.ActivationFunctionType
ALU = mybir.AluOpType


# ---------------------------------------------------------------- host prep

def _part_major(arr, ntiles, pad_val):
    """[ntiles*128] -> [128, ntiles] partition-major; pads with pad_val."""
    out = np.full(ntiles * P, pad_val, dtype=arr.dtype)
    out[: len(arr)] = arr
    return np.ascontiguousarray(out.reshape(ntiles, P).T)


def host_prep(x, edge_index, edge_attr, ptr):
    src = np.asarray(edge_index[0], dtype=np.int64)
    dst = np.asarray(edge_index[1], dtype=np.int64)
    ea = np.asarray(edge_attr, dtype=np.int64)
    ptr = np.asarray(ptr, dtype=np.int64)
    node_batch = (np.searchsorted(ptr, np.arange(N), side="right") - 1).astype(np.int64)

    # per (rel, dst) in-edge counts -> mean weights + presence
    cnt = np.zeros((R, N), dtype=np.float64)
    np.add.at(cnt, (ea, dst), 1.0)
    w_edge = (1.0 / cnt[ea, dst]).astype(np.float32)
    pres = (cnt > 0).astype(np.float32)          # [R, N]
    deg = cnt.sum(axis=0).astype(np.int64)       # in-degree per node

    # node -> (core, block, slot) rebalancing: equalize per-(core,block)
    # in-edge counts; outputs are reassembled on host, so any permutation works.
    import heapq
    ncap = np.full((C, NB), P, dtype=np.int64)
    ncap[:, NB - 1] = NSH - (NB - 1) * P
    heap = [(0, int(c), int(b)) for c in range(C) for b in range(NB)]
    heapq.heapify(heap)
    used = np.zeros((C, NB), dtype=np.int64)
    node_core = np.empty(N, dtype=np.int64)
    node_loc = np.empty(N, dtype=np.int64)
    for n in np.argsort(-deg, kind="stable"):
        while True:
            w_, c, b = heapq.heappop(heap)
            if used[c, b] < ncap[c, b]:
                break
        node_core[n] = c
        node_loc[n] = b * P + used[c, b]
        used[c, b] += 1
        if used[c, b] < ncap[c, b]:
            heapq.heappush(heap, (w_ + int(deg[n]), c, b))
    glob_of = (node_core * NSH + node_loc).astype(np.int64)  # h1full row
    core_of = node_core[dst]
    eloc = node_loc[dst]

    # ---- pass-1 schedule: relation-major buckets, padded to 128 per rel,
    # tile counts uniform across cores (max over cores per rel).
    rel_lists = []
    for c in range(C):
        eidx = np.nonzero(core_of == c)[0]
        d_loc = eloc[eidx]
        o1 = np.lexsort((d_loc, ea[eidx]))
        e1 = eidx[o1]
        rel_lists.append([e1[ea[e1] == r] for r in range(R)])
    T1_r = [max(int(np.ceil(len(rel_lists[c][r]) / P)) for c in range(C)) for r in range(R)]
    T1 = sum(T1_r)
    E1p = T1 * P

    src1 = np.zeros((C, E1p), dtype=np.int32)
    src1g = np.zeros((C, E1p), dtype=np.int32)
    w1 = np.zeros((C, E1p), dtype=np.float32)
    pos1 = [dict() for _ in range(C)]
    for c in range(C):
        off = 0
        for r in range(R):
            el = rel_lists[c][r]
            src1[c, off:off + len(el)] = src[el]
            src1g[c, off:off + len(el)] = glob_of[src[el]]
            w1[c, off:off + len(el)] = w_edge[el]
            for k, e in enumerate(el):
                pos1[c][e] = off + k
            off += T1_r[r] * P

    # ---- pass-2 schedule: dst-block-major, per-block tiles uniform across cores
    blk_lists = []
    for c in range(C):
        eidx = np.nonzero(core_of == c)[0]
        d_loc = eloc[eidx]
        o2 = np.argsort(d_loc, kind="stable")
        e2 = eidx[o2]
        blk = (d_loc[o2]) // P
        blk_lists.append([e2[blk == b] for b in range(NB)])
    T2_b = [max(int(np.ceil(len(blk_lists[c][b]) / P)) for c in range(C)) for b in range(NB)]
    T2 = sum(T2_b)
    E2p = max(T2, 1) * P

    gidx2 = np.zeros((C, E2p), dtype=np.int32)
    slot2 = np.full((C, E2p), -1.0, dtype=np.float32)
    for c in range(C):
        off = 0
        for b in range(NB):
            el = blk_lists[c][b]
            d_loc = eloc[el]
            gidx2[c, off:off + len(el)] = [pos1[c][e] for e in el]
            slot2[c, off:off + len(el)] = (d_loc % P).astype(np.float32)
            off += T2_b[b] * P

    sched = dict(T1_r=T1_r, T1=T1, T2_b=T2_b, T2=T2, E1p=E1p, E2p=E2p)
    # ---- per-node relation presence (9 x NSH, row0 = ones) per core, loc order
    paug = np.zeros((C, R + 1, NSH), dtype=np.float32)
    for c in range(C):
        paug[c, 0, :] = 1.0
        sel = node_core == c
        paug[c, 1:, node_loc[sel]] = pres[:, sel].T

    # ---- pooling / head constants (loc order)
    batchslot = np.zeros((C, P, NB), dtype=np.float32)
    for c in range(C):
        sel = node_core == c
        nb_loc = np.empty(NSH, dtype=np.float32)
        nb_loc[node_loc[sel]] = node_batch[sel].astype(np.float32)
        batchslot[c] = _part_major(nb_loc, NB, -1.0)

    cnt_gr = np.zeros((G, R), dtype=np.float64)
    np.add.at(cnt_gr, (node_batch[src], ea), 1.0)
    cnt_gr = cnt_gr.astype(np.float32)
    presneg = (-60.0 * (cnt_gr == 0)).astype(np.float32)
    nvec = cnt_gr.sum(axis=0, keepdims=True).astype(np.float32)   # [1, R]
    gsizes = np.diff(ptr).astype(np.float32)
    cnts_nodes = np.maximum(gsizes, 1.0).reshape(G, 1)

    arrays = dict(
        src1_raw=[src1[c].copy() for c in range(C)],
        node_core=node_core, node_loc=node_loc,
        src1=[_part_major(src1g[c], T1, 0) for c in range(C)],
        w1=[_part_major(w1[c], T1, 0.0) for c in range(C)],
        gidx2=[_part_major(gidx2[c], max(T2, 1), 0) for c in range(C)],
        slot2=[_part_major(slot2[c], max(T2, 1), -1.0) for c in range(C)],
        paug=[np.ascontiguousarray(paug[c]) for c in range(C)],
        batchslot=[np.ascontiguousarray(batchslot[c]) for c in range(C)],
        cnt_gr=cnt_gr, presneg=presneg, nvec=nvec, cnts_nodes=cnts_nodes,
    )
    return sched, arrays


# ---------------------------------------------------------------- builder

def build(sched, ba2_val):
    T1_r, T2_b = sched["T1_r"], sched["T2_b"]
    T1, T2 = sched["T1"], sched["T2"]
    E1p, E2p = sched["E1p"], sched["E2p"]

    nc = bacc.Bacc("TRN2", target_bir_lowering=False, debug=False, num_devices=C)

    def din(name, shape, dt=F32):
        return nc.dram_tensor(name, list(shape), dt, kind="ExternalInput").ap()

    # graph-structure / schedule inputs (per-core content)
    xg1T = din("xg1T", [H, E1p], BF16)
    xTs = din("xTs", [H, NSH], BF16)
    src1 = din("src1", [P, T1], I32)
    w1 = din("w1", [P, T1])
    gidx2 = din("gidx2", [P, max(T2, 1)], I32)
    slot2 = din("slot2", [P, max(T2, 1)])
    paug = din("paug", [R + 1, NSH])
    batchslot = din("batchslot", [P, NB])
    iota128 = din("iota128", [P, P])
    iota64 = din("iota64", [P, G])
    ident_bf = din("ident_bf", [P, P], BF16)
    ident_f32 = din("ident_f32", [P, P])
    ones1x64 = din("ones1x64", [1, G])
    cnt_gr = din("cnt_gr", [G, R])
    presneg = din("presneg", [G, R])
    nvec = din("nvec", [1, R])
    cnts_nodes = din("cnts_nodes", [G, 1])
    # weights
    KH = H // P   # 2 chunks over 256
    KL = L // P   # 6 chunks over 768
    KO = OUT // P  # 6
    W_init_d = din("W_init", [H, L])
    W_initT_d = din("W_initT", [L, H])
    b_init_col = din("b_init_col", [P, KL])
    b_init_rep8 = din("b_init_rep8", [R, L])
    W_root1_d = din("W_root1", [L, H])
    b1row = din("b1row", [1, H])
    W_rel1_d = din("W_rel1", [R, L, H])
    W_root2_d = din("W_root2", [H, H])
    b2row = din("b2row", [1, H])
    W_rel2_d = din("W_rel2", [R, H, H])
    Wg_d = din("Wg", [H, L])
    bg_col = din("bg_col", [P, KL])
    Wa1_d = din("Wa1", [L, L])
    ba1_col = din("ba1_col", [P, KL])
    Wa2_col = din("Wa2_col", [P, KL])
    rel_tableT = din("rel_tableT", [H, R])
    Wo_d = din("Wo", [2 * L, OUT])
    bo_col = din("bo_col", [P, KO])
    lng_rep = din("lng_rep", [G, OUT])
    lnb_rep = din("lnb_rep", [G, OUT])

    out_nodes = nc.dram_tensor("out_nodes", [NSH, H], F32, kind="ExternalOutput").ap()
    out_graph = nc.dram_tensor("out_graph", [G, OUT], F32, kind="ExternalOutput").ap()

    with tile.TileContext(nc) as tc:
        with (
            tc.tile_pool(name="const", bufs=1) as cp,
            tc.tile_pool(name="dram", bufs=1, space="DRAM") as dr,
        ):
            # ---------------- resident schedule arrays / constants
            src1_sb = cp.tile([P, T1], I32)
            nc.sync.dma_start(out=src1_sb[:], in_=src1[:])
            w1_sb = cp.tile([P, T1], F32)
            nc.sync.dma_start(out=w1_sb[:], in_=w1[:])
            gidx2_sb = cp.tile([P, max(T2, 1)], I32)
            nc.sync.dma_start(out=gidx2_sb[:], in_=gidx2[:])
            slot2_sb = cp.tile([P, max(T2, 1)], F32)
            nc.sync.dma_start(out=slot2_sb[:], in_=slot2[:])
            paug_sb = cp.tile([R + 1, NSH], F32)
            nc.sync.dma_start(out=paug_sb[:], in_=paug[:])
            bslot_sb = cp.tile([P, NB], F32)
            nc.sync.dma_start(out=bslot_sb[:], in_=batchslot[:])
            iota128_sb = cp.tile([P, P], F32)
            nc.sync.dma_start(out=iota128_sb[:], in_=iota128[:])
            iota64_sb = cp.tile([P, G], F32)
            nc.sync.dma_start(out=iota64_sb[:], in_=iota64[:])
            identbf_sb = cp.tile([P, P], BF16)
            nc.sync.dma_start(out=identbf_sb[:], in_=ident_bf[:])
            identf_sb = cp.tile([P, P], F32)
            nc.sync.dma_start(out=identf_sb[:], in_=ident_f32[:])

            xTs_sb = [cp.tile([P, NSH], BF16, name=f"xTs{k}") for k in range(KH)]
            for k in range(KH):
                nc.sync.dma_start(out=xTs_sb[k][:], in_=xTs[k * P:(k + 1) * P, :])
            h1T_sb = [cp.tile([P, NSH], BF16, name=f"h1T{k}") for k in range(KH)]

            # fused weights (resident, bf16)
            A1 = [[cp.tile([P, H], BF16, name=f"A1_{r}_{k}") for k in range(KH)] for r in range(R)]
            A1root = [cp.tile([P, H], BF16, name=f"A1root_{k}") for k in range(KH)]
            A2 = [[cp.tile([P, H], BF16, name=f"A2_{r}_{k}") for k in range(KH)] for r in range(R)]
            A2root = [cp.tile([P, H], BF16, name=f"A2root_{k}") for k in range(KH)]
            Caug1_sb = cp.tile([R + 1, H], F32)
            b2_sb = cp.tile([1, H], F32)
            nc.sync.dma_start(out=b2_sb[:], in_=b2row[:])
            K2Lc = 2 * L // P
            Wg_sb = [cp.tile([P, L], F32, name=f"Wg{k}") for k in range(KH)]
            for k in range(KH):
                nc.sync.dma_start(out=Wg_sb[k][:], in_=Wg_d[k * P:(k + 1) * P, :])
            bg_sb = cp.tile([P, KL], F32)
            nc.sync.dma_start(out=bg_sb[:], in_=bg_col[:])
            Wo_sb = [cp.tile([P, OUT], F32, name=f"Wo{k}") for k in range(K2Lc)]
            for k in range(K2Lc):
                nc.sync.dma_start(out=Wo_sb[k][:], in_=Wo_d[k * P:(k + 1) * P, :])
            bo_sb = cp.tile([P, KO], F32)
            nc.sync.dma_start(out=bo_sb[:], in_=bo_col[:])

            # big DRAM intermediates
            caug_dram = dr.tile([R + 1, H], F32)
            msg1 = dr.tile([E1p, H], BF16)
            msg2 = dr.tile([E1p, H], BF16)
            h1f32 = dr.tile([NSH, H], F32)
            h1bf_sh = dr.tile([NSH, H], BF16)
            h1full = dr.tile([N, H], BF16, addr_space="Shared")
            poolin = dr.tile([G, H], F32)
            poolout = dr.tile([G, H], F32, addr_space="Shared")

            # ---------------- phase W: fused weight prep
            with (
                tc.tile_pool(name="wp", bufs=2) as wp,
                tc.tile_pool(name="wpp", bufs=2, space="PSUM") as wpp,
            ):
                WiT = [wp.tile([P, H], F32, name=f"WiT{k}", bufs=1) for k in range(KL)]
                for k in range(KL):
                    nc.sync.dma_start(out=WiT[k][:], in_=W_initT_d[k * P:(k + 1) * P, :])
                binit_sb = wp.tile([P, KL], F32, bufs=1)
                nc.sync.dma_start(out=binit_sb[:], in_=b_init_col[:])

                def fuse(dst_tiles, rhs_dram, bias_row_idx):
                    """dst[mc] = (W_init @ rhs)[mc*128:(mc+1)*128, :] as bf16;
                    also Caug1[bias_row_idx] = b_init @ rhs (f32)."""
                    rhs_t = [wp.tile([P, H], F32, name="fuserhs", tag="fuserhs", bufs=3)
                             for _ in range(KL)]
                    for k in range(KL):
                        nc.sync.dma_start(out=rhs_t[k][:], in_=rhs_dram[k * P:(k + 1) * P, :])
                    for mc in range(KH):
                        ps = wpp.tile([P, H], F32, name="fusepsum", tag="fps")
                        for k in range(KL):
                            nc.tensor.matmul(
                                out=ps[:], lhsT=WiT[k][:, mc * P:(mc + 1) * P],
                                rhs=rhs_t[k][:], start=(k == 0), stop=(k == KL - 1))
                        nc.vector.tensor_copy(out=dst_tiles[mc][:], in_=ps[:])
                    psb = wpp.tile([1, H], F32, name="fusebias", tag="fpb")
                    for k in range(KL):
                        nc.tensor.matmul(out=psb[:], lhsT=binit_sb[:, k:k + 1],
                                         rhs=rhs_t[k][:], start=(k == 0), stop=(k == KL - 1))
                    stage = wp.tile([1, H], F32, name="cstage", tag="cstage", bufs=2)
                    nc.vector.tensor_copy(out=stage[:], in_=psb[:])
                    nc.sync.dma_start(
                        out=caug_dram[bias_row_idx:bias_row_idx + 1, :], in_=stage[:])

                fuse(A1root, W_root1_d, 0)
                for r in range(R):
                    fuse(A1[r], W_rel1_d[r], 1 + r)
                nc.sync.dma_start(out=Caug1_sb[:], in_=caug_dram[:])
                # Caug1 row0 += b1
                b1_sb = wp.tile([1, H], F32)
                nc.sync.dma_start(out=b1_sb[:], in_=b1row[:])
                nc.vector.tensor_add(out=Caug1_sb[0:1, :], in0=Caug1_sb[0:1, :], in1=b1_sb[:])

                # layer-2 weights: straight bf16 casts
                for r in range(R):
                    for k in range(KH):
                        t = wp.tile([P, H], F32, name="w2ld", tag="w2ld", bufs=3)
                        nc.sync.dma_start(out=t[:], in_=W_rel2_d[r][k * P:(k + 1) * P, :])
                        nc.vector.tensor_copy(out=A2[r][k][:], in_=t[:])
                for k in range(KH):
                    t = wp.tile([P, H], F32, name="w2r", tag="w2ld", bufs=3)
                    nc.sync.dma_start(out=t[:], in_=W_root2_d[k * P:(k + 1) * P, :])
                    nc.vector.tensor_copy(out=A2root[k][:], in_=t[:])

            # ---------------- message pass (shared for both layers)
            def pass1(src_dram_bf, Ar, msgbuf):
                with (
                    tc.tile_pool(name="p1", bufs=6) as sp,
                    tc.tile_pool(name="p1p", bufs=4, space="PSUM") as pp,
                ):
                    t = 0
                    for r in range(R):
                        for _ in range(T1_r[r]):
                            g = sp.tile([P, H], BF16, name="g", tag="g")
                            nc.gpsimd.indirect_dma_start(
                                out=g[:], out_offset=None, in_=src_dram_bf[:],
                                in_offset=bass.IndirectOffsetOnAxis(
                                    ap=src1_sb[:, t:t + 1], axis=0))
                            gT = []
                            for k in range(KH):
                                tp = pp.tile([P, P], BF16, name="tp", tag="tp")
                                nc.tensor.transpose(
                                    out=tp[:], in_=g[:, k * P:(k + 1) * P],
                                    identity=identbf_sb[:])
                                gk = sp.tile([P, P], BF16, name="gT", tag=f"gT{k}")
                                nc.vector.tensor_copy(out=gk[:], in_=tp[:])
                                gT.append(gk)
                            mm = pp.tile([P, H], F32, name="mm", tag="mm")
                            for k in range(KH):
                                nc.tensor.matmul(out=mm[:], lhsT=gT[k][:], rhs=Ar[r][k][:],
                                                 start=(k == 0), stop=(k == KH - 1))
                            mb = sp.tile([P, H], BF16, name="mb", tag="mb")
                            nc.vector.tensor_scalar_mul(mb[:], mm[:], w1_sb[:, t:t + 1])
                            nc.sync.dma_start(out=msgbuf[t * P:(t + 1) * P, :], in_=mb[:])
                            t += 1

            def pass2(msgbuf, Aroot, lhsT_sb, caug_k, on_block):
                with (
                    tc.tile_pool(name="p2", bufs=8) as sp,
                    tc.tile_pool(name="p2p", bufs=4, space="PSUM") as pp,
                ):
                    t = 0
                    for b in range(NB):
                        nb_sz = min(P, NSH - b * P)
                        acc = pp.tile([P, H], F32, name="acc", tag="acc")
                        for k in range(KH):
                            nc.tensor.matmul(
                                out=acc[:nb_sz], lhsT=lhsT_sb[k][:, b * P:b * P + nb_sz],
                                rhs=Aroot[k][:], start=(k == 0), stop=False)
                        nc.tensor.matmul(
                            out=acc[:nb_sz], lhsT=paug_sb[:caug_k, b * P:b * P + nb_sz],
                            rhs=(Caug1_sb[:caug_k, :] if caug_k > 1 else b2_sb[:]),
                            start=False, stop=(T2_b[b] == 0))
                        for _ in range(T2_b[b]):
                            mg = sp.tile([P, H], BF16, name="mg", tag="mg")
                            nc.gpsimd.indirect_dma_start(
                                out=mg[:], out_offset=None, in_=msgbuf[:],
                                in_offset=bass.IndirectOffsetOnAxis(
                                    ap=gidx2_sb[:, t:t + 1], axis=0))
                            oh = sp.tile([P, P], BF16, name="oh", tag="oh")
                            nc.vector.tensor_tensor(
                                out=oh[:], in0=slot2_sb[:, t:t + 1].to_broadcast([P, P]),
                                in1=iota128_sb[:], op=ALU.is_equal)
                            last = t == sum(T2_b[:b + 1]) - 1
                            nc.tensor.matmul(out=acc[:nb_sz], lhsT=oh[:, :nb_sz], rhs=mg[:],
                                             start=False, stop=last)
                            t += 1
                        on_block(b, nb_sz, acc, sp, pp)

            # ---------------- layer 1: host-pregathered feature-major stage;
            # 64-padded runs write msg rows directly at partition offsets.
            def pass1_direct(Ar, msgbuf):
                GB = 8
                rels = [r for r in range(R) for _ in range(T1_r[r])]
                with (
                    tc.tile_pool(name="p1d", bufs=4) as sp,
                    tc.tile_pool(name="p1dp", bufs=4, space="PSUM") as pp,
                ):
                    for g0 in range(0, T1, GB):
                        k_gr = min(GB, T1 - g0)
                        gw = [sp.tile([P, GB * P], BF16, name=f"gw{k}",
                                      tag=f"gw{k}") for k in range(KH)]
                        for k in range(KH):
                            nc.sync.dma_start(
                                out=gw[k][:, :k_gr * P],
                                in_=xg1T[k * P:(k + 1) * P, g0 * P:(g0 + k_gr) * P])
                        for j in range(k_gr):
                            t = g0 + j
                            mm = pp.tile([P, H], F32, name="mmd", tag="mmd")
                            for k in range(KH):
                                nc.tensor.matmul(
                                    out=mm[:], lhsT=gw[k][:, j * P:(j + 1) * P],
                                    rhs=Ar[rels[t]][k][:],
                                    start=(k == 0), stop=(k == KH - 1))
                            mb = sp.tile([P, H], BF16, name="mbd", tag="mbd")
                            nc.vector.tensor_scalar_mul(mb[:], mm[:], w1_sb[:, t:t + 1])
                            nc.sync.dma_start(out=msgbuf[t * P:(t + 1) * P, :],
                                              in_=mb[:])

            def l1_block(b, nb_sz, acc, sp, pp):
                h1b = sp.tile([P, H], F32, name="h1b", tag="h1b")
                nc.scalar.activation(out=h1b[:nb_sz], in_=acc[:nb_sz], func=AF.Relu)
                hbf = sp.tile([P, H], BF16, name="hbf", tag="hbf")
                nc.vector.tensor_copy(out=hbf[:nb_sz], in_=h1b[:nb_sz])
                nc.sync.dma_start(out=h1f32[b * P:b * P + nb_sz, :], in_=h1b[:nb_sz])
                nc.sync.dma_start(out=h1bf_sh[b * P:b * P + nb_sz, :], in_=hbf[:nb_sz])
                for k in range(KH):
                    tp = pp.tile([P, P], BF16, name="tph", tag="tph")
                    nc.tensor.transpose(out=tp[:, :nb_sz], in_=hbf[:nb_sz, k * P:(k + 1) * P],
                                        identity=identbf_sb[:nb_sz, :nb_sz])
                    nc.vector.tensor_copy(out=h1T_sb[k][:, b * P:b * P + nb_sz],
                                          in_=tp[:, :nb_sz])

            # ---------------- early head (pooling-independent attention branch)
            rcT = [cp.tile([P, G], F32, name=f"rcT{m}") for m in range(KL)]
            with (
                tc.tile_pool(name="eh", bufs=2) as hp,
                tc.tile_pool(name="ehp", bufs=2, space="PSUM") as hpp,
            ):
                def hps(p_, f_):
                    return hpp.tile([p_, f_], F32, name="ehps", tag="ehps")

                # rel8 [R, L] = rel_table @ W_init + b_init
                rtT = [hp.tile([P, R], F32, name=f"rtT{k}", bufs=1) for k in range(KH)]
                for k in range(KH):
                    nc.sync.dma_start(out=rtT[k][:], in_=rel_tableT[k * P:(k + 1) * P, :])
                Wi_sb = [hp.tile([P, L], F32, name=f"Wi{k}", bufs=1) for k in range(KH)]
                for k in range(KH):
                    nc.sync.dma_start(out=Wi_sb[k][:], in_=W_init_d[k * P:(k + 1) * P, :])
                rel8 = hp.tile([R, L], F32, bufs=1)
                for half in range(2):
                    sl = slice(half * (L // 2), (half + 1) * (L // 2))
                    ps = hps(R, L // 2)
                    for k in range(KH):
                        nc.tensor.matmul(out=ps[:], lhsT=rtT[k][:],
                                         rhs=Wi_sb[k][:, sl], start=(k == 0), stop=(k == KH - 1))
                    nc.vector.tensor_copy(out=rel8[:, sl], in_=ps[:])
                bi8 = hp.tile([R, L], F32, bufs=1)
                nc.sync.dma_start(out=bi8[:], in_=b_init_rep8[:])
                nc.vector.tensor_add(out=rel8[:], in0=rel8[:], in1=bi8[:])
                # rel8^T [L, R]
                r8T = [hp.tile([P, R], F32, name=f"r8T{k}", bufs=1) for k in range(KL)]
                for k in range(KL):
                    tp = hps(P, R)
                    nc.tensor.transpose(out=tp[:, :R], in_=rel8[:, k * P:(k + 1) * P],
                                        identity=identf_sb[:R, :R])
                    nc.vector.tensor_copy(out=r8T[k][:], in_=tp[:, :R])
                # t1T = tanh(Wa1^T @ rel8^T + ba1) [L, R]
                Wa1_sb = [hp.tile([P, L], F32, name=f"Wa1{k}", bufs=1) for k in range(KL)]
                for k in range(KL):
                    nc.sync.dma_start(out=Wa1_sb[k][:], in_=Wa1_d[k * P:(k + 1) * P, :])
                ba1_sb = hp.tile([P, KL], F32, bufs=1)
                nc.sync.dma_start(out=ba1_sb[:], in_=ba1_col[:])
                t1T = [hp.tile([P, R], F32, name=f"t1T{m}", bufs=1) for m in range(KL)]
                for m in range(KL):
                    ps = hps(P, R)
                    for k in range(KL):
                        nc.tensor.matmul(out=ps[:], lhsT=Wa1_sb[k][:, m * P:(m + 1) * P],
                                         rhs=r8T[k][:], start=(k == 0), stop=(k == KL - 1))
                    nc.scalar.activation(out=t1T[m][:], in_=ps[:], func=AF.Tanh,
                                         bias=ba1_sb[:, m:m + 1])
                # s8 [1, R]
                Wa2_sb = hp.tile([P, KL], F32, bufs=1)
                nc.sync.dma_start(out=Wa2_sb[:], in_=Wa2_col[:])
                ps8 = hps(1, R)
                for k in range(KL):
                    nc.tensor.matmul(out=ps8[:], lhsT=Wa2_sb[:, k:k + 1],
                                     rhs=t1T[k][:], start=(k == 0), stop=(k == KL - 1))
                s8 = hp.tile([1, R], F32, bufs=1)
                nc.scalar.add(out=s8[:], in_=ps8[:], add=float(ba2_val))
                # global softmax with counts
                smax = hp.tile([1, 1], F32, bufs=1)
                nc.vector.tensor_reduce(out=smax[:], in_=s8[:], axis=mybir.AxisListType.X,
                                        op=ALU.max)
                nsmax = hp.tile([1, 1], F32, bufs=1)
                nc.vector.tensor_scalar_mul(nsmax[:], smax[:], -1.0)
                e8 = hp.tile([1, R], F32, bufs=1)
                nc.scalar.activation(out=e8[:], in_=s8[:], func=AF.Exp, bias=nsmax[:, :1])
                nv_sb = hp.tile([1, R], F32, bufs=1)
                nc.sync.dma_start(out=nv_sb[:], in_=nvec[:])
                wsum = hp.tile([1, R], F32, bufs=1)
                nc.vector.tensor_mul(out=wsum[:], in0=e8[:], in1=nv_sb[:])
                den = hp.tile([1, 1], F32, bufs=1)
                nc.vector.tensor_reduce(out=den[:], in_=wsum[:], axis=mybir.AxisListType.X,
                                        op=ALU.add)
                rden = hp.tile([1, 1], F32, bufs=1)
                nc.vector.reciprocal(out=rden[:], in_=den[:])
                aval = hp.tile([1, R], F32, bufs=1)
                nc.vector.tensor_scalar_mul(aval[:], e8[:], rden[:, :1])
                # replicate aval over 64 partitions
                ones64_sb = hp.tile([1, G], F32, bufs=1)
                nc.sync.dma_start(out=ones64_sb[:], in_=ones1x64[:])
                avps = hps(G, R)
                nc.tensor.matmul(out=avps[:], lhsT=ones64_sb[:], rhs=aval[:],
                                 start=True, stop=True)
                avrep = hp.tile([G, R], F32, bufs=1)
                nc.vector.tensor_copy(out=avrep[:], in_=avps[:])
                # segment softmax via counts
                cg_sb = hp.tile([G, R], F32, bufs=1)
                nc.sync.dma_start(out=cg_sb[:], in_=cnt_gr[:])
                pn_sb = hp.tile([G, R], F32, bufs=1)
                nc.sync.dma_start(out=pn_sb[:], in_=presneg[:])
                gmin = hp.tile([G, R], F32, bufs=1)
                nc.vector.tensor_add(out=gmin[:], in0=avrep[:], in1=pn_sb[:])
                gmax = hp.tile([G, 1], F32, bufs=1)
                nc.vector.tensor_reduce(out=gmax[:], in_=gmin[:], axis=mybir.AxisListType.X,
                                        op=ALU.max)
                ngmax = hp.tile([G, 1], F32, bufs=1)
                nc.vector.tensor_scalar_mul(ngmax[:], gmax[:], -1.0)
                eg = hp.tile([G, R], F32, bufs=1)
                nc.scalar.activation(out=eg[:], in_=avrep[:], func=AF.Exp, bias=ngmax[:, :1])
                wden = hp.tile([G, R], F32, bufs=1)
                nc.vector.tensor_mul(out=wden[:], in0=eg[:], in1=cg_sb[:])
                deng = hp.tile([G, 1], F32, bufs=1)
                nc.vector.tensor_reduce(out=deng[:], in_=wden[:], axis=mybir.AxisListType.X,
                                        op=ALU.add)
                dengc = hp.tile([G, 1], F32, bufs=1)
                nc.vector.tensor_scalar_max(dengc[:], deng[:], 1e-30)
                rdeng = hp.tile([G, 1], F32, bufs=1)
                nc.vector.reciprocal(out=rdeng[:], in_=dengc[:])
                coef = hp.tile([G, R], F32, bufs=1)
                nc.vector.tensor_scalar_mul(coef[:], wden[:], rdeng[:, :1])
                # coef^T [R, G]
                cps = hps(R, G)
                nc.tensor.transpose(out=cps[:R, :], in_=coef[:, :R],
                                    identity=identf_sb[:G, :G])
                coefT = hp.tile([R, G], F32, bufs=1)
                nc.vector.tensor_copy(out=coefT[:], in_=cps[:R, :])
                # rel_ctx^T [L, G] = rel8^T @ coef^T
                for m in range(KL):
                    ps = hps(P, G)
                    nc.tensor.matmul(out=ps[:], lhsT=rel8[:, m * P:(m + 1) * P],
                                     rhs=coefT[:], start=True, stop=True)
                    nc.vector.tensor_copy(out=rcT[m][:], in_=ps[:])

            pass1_direct(A1, msg1)
            pass2(msg1, A1root, xTs_sb, R + 1, l1_block)

            # ---------------- AllGather h1 (bf16)
            nc.gpsimd.collective_compute(
                "AllGather", ALU.bypass,
                ins=[h1bf_sh.opt()], outs=[h1full.opt()],
                replica_groups=[list(range(C))])

            # ---------------- layer 2 (+ residual + pooling partials)
            poolsum_sb = cp.tile([G, H], F32)
            nc.vector.memset(poolsum_sb[:], 0.0)

            def l2_block(b, nb_sz, acc, sp, pp):
                t = sp.tile([P, H], F32, name="t2", tag="t2")
                nc.scalar.activation(out=t[:nb_sz], in_=acc[:nb_sz], func=AF.Relu)
                h1l = sp.tile([P, H], F32, name="h1l", tag="h1l")
                nc.sync.dma_start(out=h1l[:nb_sz], in_=h1f32[b * P:b * P + nb_sz, :])
                h2 = sp.tile([P, H], F32, name="h2", tag="h2")
                if nb_sz < P:
                    nc.vector.memset(h2[:], 0.0)
                nc.vector.tensor_add(out=h2[:nb_sz], in0=t[:nb_sz], in1=h1l[:nb_sz])
                nc.sync.dma_start(out=out_nodes[b * P:b * P + nb_sz, :], in_=h2[:nb_sz])
                oh64 = sp.tile([P, G], F32, name="oh64", tag="oh64")
                nc.vector.tensor_tensor(
                    out=oh64[:], in0=bslot_sb[:, b:b + 1].to_broadcast([P, G]),
                    in1=iota64_sb[:], op=ALU.is_equal)
                pps = pp.tile([G, H], F32, name="pps", tag="pps")
                nc.tensor.matmul(out=pps[:], lhsT=oh64[:], rhs=h2[:], start=True, stop=True)
                nc.vector.tensor_add(out=poolsum_sb[:], in0=poolsum_sb[:], in1=pps[:])

            pass1(h1full, A2, msg2)
            pass2(msg2, A2root, h1T_sb, 1, l2_block)

            # ---------------- AllReduce pooled sums
            nc.sync.dma_start(out=poolin[:], in_=poolsum_sb[:])
            nc.gpsimd.collective_compute(
                "AllReduce", ALU.add,
                ins=[poolin.opt()], outs=[poolout.opt()],
                replica_groups=[list(range(C))])

            # ---------------- head
            with (
                tc.tile_pool(name="hd", bufs=2) as hp,
                tc.tile_pool(name="hdp", bufs=2, space="PSUM") as hpp,
            ):
                K2L = 2 * L // P  # 12

                def hps(p_, f_):
                    return hpp.tile([p_, f_], F32, name="hps", tag="hps")

                # pooled mean [G, H]
                sums = hp.tile([G, H], F32, bufs=1)
                nc.sync.dma_start(out=sums[:], in_=poolout[:])
                cnts_sb = hp.tile([G, 1], F32, bufs=1)
                nc.sync.dma_start(out=cnts_sb[:], in_=cnts_nodes[:])
                rc = hp.tile([G, 1], F32, bufs=1)
                nc.vector.reciprocal(out=rc[:], in_=cnts_sb[:])
                mp = hp.tile([G, H], F32, bufs=1)
                nc.vector.tensor_scalar_mul(mp[:], sums[:], rc[:, :1])
                # meanpool^T [H, G]
                mpT = [hp.tile([P, G], F32, name=f"mpT{k}", bufs=1) for k in range(KH)]
                for k in range(KH):
                    tp = hps(P, G)
                    nc.tensor.transpose(out=tp[:], in_=mp[:, k * P:(k + 1) * P],
                                        identity=identf_sb[:G, :G])
                    nc.vector.tensor_copy(out=mpT[k][:], in_=tp[:])
                # graph_emb^T [L, G] = Wg^T @ mp^T + bg
                geT = [hp.tile([P, G], F32, name=f"geT{m}", bufs=1) for m in range(KL)]
                for m in range(KL):
                    ps = hps(P, G)
                    for k in range(KH):
                        nc.tensor.matmul(out=ps[:], lhsT=Wg_sb[k][:, m * P:(m + 1) * P],
                                         rhs=mpT[k][:], start=(k == 0), stop=(k == KH - 1))
                    nc.vector.tensor_scalar_add(geT[m][:], ps[:], bg_sb[:, m:m + 1])

                # comb^T [OUT, G] = Wo^T @ [geT; rcT] + bo
                cat = geT + rcT
                combT = [hp.tile([P, G], F32, name=f"combT{m}", bufs=1) for m in range(KO)]
                for m in range(KO):
                    ps = hps(P, G)
                    for k in range(K2L):
                        nc.tensor.matmul(out=ps[:], lhsT=Wo_sb[k][:, m * P:(m + 1) * P],
                                         rhs=cat[k][:], start=(k == 0), stop=(k == K2L - 1))
                    nc.vector.tensor_scalar_add(combT[m][:], ps[:], bo_sb[:, m:m + 1])
                # transpose back to [G, OUT]
                comb = hp.tile([G, OUT], F32, bufs=1)
                for m in range(KO):
                    tp = hps(G, P)
                    nc.tensor.transpose(out=tp[:G, :], in_=combT[m][:, :G],
                                        identity=identf_sb[:])
                    nc.vector.tensor_copy(out=comb[:, m * P:(m + 1) * P], in_=tp[:G, :])
                # layernorm over OUT
                nmu = hp.tile([G, 1], F32, bufs=1)
                nc.vector.tensor_reduce(out=nmu[:], in_=comb[:], axis=mybir.AxisListType.X,
                                        op=ALU.add)
                nc.vector.tensor_scalar_mul(nmu[:], nmu[:], -1.0 / OUT)
                xc = hp.tile([G, OUT], F32, bufs=1)
                nc.vector.tensor_scalar_add(xc[:], comb[:], nmu[:, :1])
                sq = hp.tile([G, OUT], F32, bufs=1)
                vsum = hp.tile([G, 1], F32, bufs=1)
                nc.scalar.activation(out=sq[:], in_=xc[:], func=AF.Square, accum_out=vsum[:])
                var = hp.tile([G, 1], F32, bufs=1)
                nc.vector.tensor_scalar(out=var[:], in0=vsum[:], scalar1=1.0 / OUT,
                                        scalar2=1e-5, op0=ALU.mult, op1=ALU.add)
                sd = hp.tile([G, 1], F32, bufs=1)
                nc.scalar.activation(out=sd[:], in_=var[:], func=AF.Sqrt)
                rsd = hp.tile([G, 1], F32, bufs=1)
                nc.vector.reciprocal(out=rsd[:], in_=sd[:])
                y = hp.tile([G, OUT], F32, bufs=1)
                nc.vector.tensor_scalar_mul(y[:], xc[:], rsd[:, :1])
                lg = hp.tile([G, OUT], F32, bufs=1)
                nc.sync.dma_start(out=lg[:], in_=lng_rep[:])
                lb = hp.tile([G, OUT], F32, bufs=1)
                nc.sync.dma_start(out=lb[:], in_=lnb_rep[:])
                nc.vector.tensor_mul(out=y[:], in0=y[:], in1=lg[:])
                nc.vector.tensor_add(out=y[:], in0=y[:], in1=lb[:])
                nc.sync.dma_start(out=out_graph[:], in_=y[:])

    nc.compile()
    return nc


# ---------------------------------------------------------------- runner

_CACHE = {}


def kernel(x, edge_index, edge_attr, batch, ptr, W_init, b_init, rel_table,
           W_root1, W_rel1, b1, W_root2, W_rel2, b2, Wg, bg, Wa1, ba1, Wa2, ba2,
           Wo, bo, ln_g, ln_b, _run_kwargs=None):
    x = np.asarray(x, dtype=np.float32)
    sched, arr = host_prep(x, np.asarray(edge_index), np.asarray(edge_attr),
                           np.asarray(ptr))

    key = (tuple(sched["T1_r"]), tuple(sched["T2_b"]),
           float(np.asarray(ba2).ravel()[0]))
    if key not in _CACHE:
        _CACHE.clear()
        _CACHE[key] = build(sched, float(np.asarray(ba2).ravel()[0]))
    nc = _CACHE[key]

    f32c = lambda a: np.ascontiguousarray(np.asarray(a, dtype=np.float32))
    iota128 = np.tile(np.arange(P, dtype=np.float32), (P, 1))
    iota64 = np.tile(np.arange(G, dtype=np.float32), (P, 1))
    ident = np.eye(P, dtype=np.float32)

    common = dict(
        iota128=iota128, iota64=iota64,
        ident_bf=ident.astype(bf16), ident_f32=ident,
        ones1x64=np.ones((1, G), np.float32),
        cnt_gr=arr["cnt_gr"], presneg=arr["presneg"], nvec=arr["nvec"],
        cnts_nodes=arr["cnts_nodes"],
        W_init=f32c(W_init), W_initT=f32c(np.asarray(W_init).T),
        b_init_col=np.ascontiguousarray(f32c(b_init).reshape(L // P, P).T),
        b_init_rep8=np.tile(f32c(b_init).reshape(1, L), (R, 1)),
        W_root1=f32c(W_root1), b1row=f32c(b1).reshape(1, H),
        W_rel1=f32c(W_rel1), W_root2=f32c(W_root2), b2row=f32c(b2).reshape(1, H),
        W_rel2=f32c(W_rel2), Wg=f32c(Wg),
        bg_col=np.ascontiguousarray(f32c(bg).reshape(L // P, P).T),
        Wa1=f32c(Wa1),
        ba1_col=np.ascontiguousarray(f32c(ba1).reshape(L // P, P).T),
        Wa2_col=np.ascontiguousarray(f32c(Wa2).reshape(L // P, P).T),
        rel_tableT=f32c(np.asarray(rel_table).T),
        Wo=f32c(Wo),
        bo_col=np.ascontiguousarray(f32c(bo).reshape(OUT // P, P).T),
        lng_rep=np.tile(f32c(ln_g).reshape(1, OUT), (G, 1)),
        lnb_rep=np.tile(f32c(ln_b).reshape(1, OUT), (G, 1)),
    )
    in_maps = []
    node_core, node_loc = arr["node_core"], arr["node_loc"]
    for c in range(C):
        m = dict(common)
        m["xg1T"] = np.ascontiguousarray(x[arr["src1_raw"][c]].T).astype(bf16)
        sel = node_core == c
        xc = np.zeros((NSH, H), dtype=np.float32)
        xc[node_loc[sel]] = x[sel]
        m["xTs"] = np.ascontiguousarray(xc.T).astype(bf16)
        m["src1"] = arr["src1"][c]
        m["w1"] = arr["w1"][c]
        m["gidx2"] = arr["gidx2"][c]
        m["slot2"] = arr["slot2"][c]
        m["paug"] = arr["paug"][c]
        m["batchslot"] = arr["batchslot"][c]
        in_maps.append(m)

    kw = _run_kwargs or {}
    res = run_bass_kernel_spmd(nc, in_maps, core_ids=list(range(C)), **kw)
    kernel._last_result = res
    node_emb = np.empty((N, H), dtype=np.float32)
    for c in range(C):
        sel = node_core == c
        node_emb[sel] = res.results[c]["out_nodes"][node_loc[sel]]
    out_g = res.results[0]["out_graph"]
    return node_emb, out_g


# revision 62
# speedup vs baseline: 1.0526x; 1.0340x over previous
"""Trainium2 Bass kernel for nn_EnhancedGraphEncoder (RGCN x2 + pooled attention head).

Self-contained: builds and runs an 8-core SPMD Bass kernel.
Sharding: nodes split into 8 equal contiguous ranges (6250 each); edges assigned
to the core owning their dst node. Weights replicated. h1 exchanged between the
two RGCN layers with an on-device AllGather (bf16).

Key algebraic restructurings (exact math, different association):
  - h0 = x@W_init + b_init is never materialized: layer-1 msg/root weights are
    fused:  A1_r = W_init @ W_rel1[r],  A1_root = W_init @ W_root1,
    bias rows c1_r = b_init @ W_rel1[r] folded in via a per-node
    relation-presence matmul (sum_e w_e c_{r_e} = sum_{r present at dst} c_r,
    since per-(rel,dst) mean weights w_e sum to 1).
  - per-edge mean normalization w_e = 1/cnt_{rel,dst} precomputed on host
    (pure graph structure), applied as per-partition scale on messages.
  - the edge-attention head collapses to per-(graph,relation) counts:
    rel_e rows take only 8 distinct values.
"""

import sys
import numpy as np

sys.path.insert(0, "/opt/trn_rl_repo")

import concourse.bass as bass
import concourse.bacc as bacc
import concourse.tile as tile
import concourse.mybir as mybir
from concourse.bass_utils import run_bass_kernel_spmd
import ml_dtypes

bf16 = ml_dtypes.bfloat16

N, E, R, H, L, OUT, G = 50000, 100000, 8, 256, 768, 768, 64
C = 8            # cores
P = 128
NSH = N // C     # 6250 nodes per core
NB = (NSH + P - 1) // P   # 49 blocks per core

F32 = mybir.dt.float32
BF16 = mybir.dt.bfloat16
I32 = mybir.dt.int32

AF = mybir.ActivationFunctionType
ALU = mybir.AluOpType


# ---------------------------------------------------------------- host prep

def _part_major(arr, ntiles, pad_val):
    """[ntiles*128] -> [128, ntiles] partition-major; pads with pad_val."""
    out = np.full(ntiles * P, pad_val, dtype=arr.dtype)
    out[: len(arr)] = arr
    return np.ascontiguousarray(out.reshape(ntiles, P).T)


def host_prep(x, edge_index, edge_attr, ptr):
    src = np.asarray(edge_index[0], dtype=np.int64)
    dst = np.asarray(edge_index[1], dtype=np.int64)
    ea = np.asarray(edge_attr, dtype=np.int64)
    ptr = np.asarray(ptr, dtype=np.int64)
    node_batch = (np.searchsorted(ptr, np.arange(N), side="right") - 1).astype(np.int64)

    # per (rel, dst) in-edge counts -> mean weights + presence
    cnt = np.zeros((R, N), dtype=np.float64)
    np.add.at(cnt, (ea, dst), 1.0)
    w_edge = (1.0 / cnt[ea, dst]).astype(np.float32)
    pres = (cnt > 0).astype(np.float32)          # [R, N]
    deg = cnt.sum(axis=0).astype(np.int64)       # in-degree per node

    # node -> (core, block, slot) rebalancing: equalize per-(core,block)
    # in-edge counts; outputs are reassembled on host, so any permutation works.
    import heapq
    ncap = np.full((C, NB), P, dtype=np.int64)
    ncap[:, NB - 1] = NSH - (NB - 1) * P
    heap = [(0, int(c), int(b)) for c in range(C) for b in range(NB)]
    heapq.heapify(heap)
    used = np.zeros((C, NB), dtype=np.int64)
    node_core = np.empty(N, dtype=np.int64)
    node_loc = np.empty(N, dtype=np.int64)
    for n in np.argsort(-deg, kind="stable"):
        while True:
            w_, c, b = heapq.heappop(heap)
            if used[c, b] < ncap[c, b]:
                break
        node_core[n] = c
        node_loc[n] = b * P + used[c, b]
        used[c, b] += 1
        if used[c, b] < ncap[c, b]:
            heapq.heappush(heap, (w_ + int(deg[n]), c, b))
    glob_of = (node_core * NSH + node_loc).astype(np.int64)  # h1full row
    core_of = node_core[dst]
    eloc = node_loc[dst]

    # ---- pass-1 schedule: relation-major buckets, padded to 128 per rel,
    # tile counts uniform across cores (max over cores per rel).
    rel_lists = []
    for c in range(C):
        eidx = np.nonzero(core_of == c)[0]
        d_loc = eloc[eidx]
        o1 = np.lexsort((d_loc, ea[eidx]))
        e1 = eidx[o1]
        rel_lists.append([e1[ea[e1] == r] for r in range(R)])
    T1_r = [max(int(np.ceil(len(rel_lists[c][r]) / P)) for c in range(C)) for r in range(R)]
    T1 = sum(T1_r)
    E1p = T1 * P

    src1 = np.zeros((C, E1p), dtype=np.int32)
    src1g = np.zeros((C, E1p), dtype=np.int32)
    w1 = np.zeros((C, E1p), dtype=np.float32)
    pos1 = [dict() for _ in range(C)]
    for c in range(C):
        off = 0
        for r in range(R):
            el = rel_lists[c][r]
            src1[c, off:off + len(el)] = src[el]
            src1g[c, off:off + len(el)] = glob_of[src[el]]
            w1[c, off:off + len(el)] = w_edge[el]
            for k, e in enumerate(el):
                pos1[c][e] = off + k
            off += T1_r[r] * P

    # ---- pass-2 schedule: dst-block-major, per-block tiles uniform across cores
    blk_lists = []
    for c in range(C):
        eidx = np.nonzero(core_of == c)[0]
        d_loc = eloc[eidx]
        o2 = np.argsort(d_loc, kind="stable")
        e2 = eidx[o2]
        blk = (d_loc[o2]) // P
        blk_lists.append([e2[blk == b] for b in range(NB)])
    T2_b = [max(int(np.ceil(len(blk_lists[c][b]) / P)) for c in range(C)) for b in range(NB)]
    T2 = sum(T2_b)
    E2p = max(T2, 1) * P

    gidx2 = np.zeros((C, E2p), dtype=np.int32)
    slot2 = np.full((C, E2p), -1.0, dtype=np.float32)
    for c in range(C):
        off = 0
        for b in range(NB):
            el = blk_lists[c][b]
            d_loc = eloc[el]
            gidx2[c, off:off + len(el)] = [pos1[c][e] for e in el]
            slot2[c, off:off + len(el)] = (d_loc % P).astype(np.float32)
            off += T2_b[b] * P

    sched = dict(T1_r=T1_r, T1=T1, T2_b=T2_b, T2=T2, E1p=E1p, E2p=E2p)
    # ---- per-node relation presence (9 x NSH, row0 = ones) per core, loc order
    paug = np.zeros((C, R + 1, NSH), dtype=np.float32)
    for c in range(C):
        paug[c, 0, :] = 1.0
        sel = node_core == c
        paug[c, 1:, node_loc[sel]] = pres[:, sel].T

    # ---- pooling / head constants (loc order)
    batchslot = np.zeros((C, P, NB), dtype=np.float32)
    for c in range(C):
        sel = node_core == c
        nb_loc = np.empty(NSH, dtype=np.float32)
        nb_loc[node_loc[sel]] = node_batch[sel].astype(np.float32)
        batchslot[c] = _part_major(nb_loc, NB, -1.0)

    cnt_gr = np.zeros((G, R), dtype=np.float64)
    np.add.at(cnt_gr, (node_batch[src], ea), 1.0)
    cnt_gr = cnt_gr.astype(np.float32)
    presneg = (-60.0 * (cnt_gr == 0)).astype(np.float32)
    nvec = cnt_gr.sum(axis=0, keepdims=True).astype(np.float32)   # [1, R]
    gsizes = np.diff(ptr).astype(np.float32)
    cnts_nodes = np.maximum(gsizes, 1.0).reshape(G, 1)

    arrays = dict(
        src1_raw=[src1[c].copy() for c in range(C)],
        node_core=node_core, node_loc=node_loc,
        src1=[_part_major(src1g[c], T1, 0) for c in range(C)],
        w1=[_part_major(w1[c], T1, 0.0) for c in range(C)],
        gidx2=[_part_major(gidx2[c], max(T2, 1), 0) for c in range(C)],
        slot2=[_part_major(slot2[c], max(T2, 1), -1.0) for c in range(C)],
        paug=[np.ascontiguousarray(paug[c]) for c in range(C)],
        batchslot=[np.ascontiguousarray(batchslot[c]) for c in range(C)],
        cnt_gr=cnt_gr, presneg=presneg, nvec=nvec, cnts_nodes=cnts_nodes,
    )
    return sched, arrays


# ---------------------------------------------------------------- builder

def build(sched, ba2_val):
    T1_r, T2_b = sched["T1_r"], sched["T2_b"]
    T1, T2 = sched["T1"], sched["T2"]
    E1p, E2p = sched["E1p"], sched["E2p"]

    nc = bacc.Bacc("TRN2", target_bir_lowering=False, debug=False, num_devices=C)

    def din(name, shape, dt=F32):
        return nc.dram_tensor(name, list(shape), dt, kind="ExternalInput").ap()

    # graph-structure / schedule inputs (per-core content)
    xg1T = din("xg1T", [H, E1p], BF16)
    xTs = din("xTs", [H, NSH], BF16)
    src1 = din("src1", [P, T1], I32)
    w1 = din("w1", [P, T1])
    gidx2 = din("gidx2", [P, max(T2, 1)], I32)
    slot2 = din("slot2", [P, max(T2, 1)])
    paug = din("paug", [R + 1, NSH])
    batchslot = din("batchslot", [P, NB])
    iota128 = din("iota128", [P, P])
    iota64 = din("iota64", [P, G])
    ident_bf = din("ident_bf", [P, P], BF16)
    ident_f32 = din("ident_f32", [P, P])
    ones1x64 = din("ones1x64", [1, G])
    cnt_gr = din("cnt_gr", [G, R])
    presneg = din("presneg", [G, R])
    nvec = din("nvec", [1, R])
    cnts_nodes = din("cnts_nodes", [G, 1])
    # weights
    KH = H // P   # 2 chunks over 256
    KL = L // P   # 6 chunks over 768
    KO = OUT // P  # 6
    W_init_d = din("W_init", [H, L])
    W_initT_d = din("W_initT", [L, H])
    b_init_col = din("b_init_col", [P, KL])
    b_init_rep8 = din("b_init_rep8", [R, L])
    W_root1_d = din("W_root1", [L, H])
    b1row = din("b1row", [1, H])
    W_rel1_d = din("W_rel1", [R, L, H])
    W_root2_d = din("W_root2", [H, H])
    b2row = din("b2row", [1, H])
    W_rel2_d = din("W_rel2", [R, H, H])
    Wg_d = din("Wg", [H, L])
    bg_col = din("bg_col", [P, KL])
    Wa1_d = din("Wa1", [L, L])
    ba1_col = din("ba1_col", [P, KL])
    Wa2_col = din("Wa2_col", [P, KL])
    rel_tableT = din("rel_tableT", [H, R])
    Wo_d = din("Wo", [2 * L, OUT])
    bo_col = din("bo_col", [P, KO])
    lng_rep = din("lng_rep", [G, OUT])
    lnb_rep = din("lnb_rep", [G, OUT])

    out_nodes = nc.dram_tensor("out_nodes", [NSH, H], F32, kind="ExternalOutput").ap()
    out_graph = nc.dram_tensor("out_graph", [G, OUT], F32, kind="ExternalOutput").ap()

    with tile.TileContext(nc) as tc:
        with (
            tc.tile_pool(name="const", bufs=1) as cp,
            tc.tile_pool(name="dram", bufs=1, space="DRAM") as dr,
        ):
            # ---------------- resident schedule arrays / constants
            src1_sb = cp.tile([P, T1], I32)
            nc.sync.dma_start(out=src1_sb[:], in_=src1[:])
            w1_sb = cp.tile([P, T1], F32)
            nc.sync.dma_start(out=w1_sb[:], in_=w1[:])
            gidx2_sb = cp.tile([P, max(T2, 1)], I32)
            nc.sync.dma_start(out=gidx2_sb[:], in_=gidx2[:])
            slot2_sb = cp.tile([P, max(T2, 1)], F32)
            nc.sync.dma_start(out=slot2_sb[:], in_=slot2[:])
            paug_sb = cp.tile([R + 1, NSH], F32)
            nc.sync.dma_start(out=paug_sb[:], in_=paug[:])
            bslot_sb = cp.tile([P, NB], F32)
            nc.sync.dma_start(out=bslot_sb[:], in_=batchslot[:])
            iota128_sb = cp.tile([P, P], F32)
            nc.sync.dma_start(out=iota128_sb[:], in_=iota128[:])
            iota64_sb = cp.tile([P, G], F32)
            nc.sync.dma_start(out=iota64_sb[:], in_=iota64[:])
            identbf_sb = cp.tile([P, P], BF16)
            nc.sync.dma_start(out=identbf_sb[:], in_=ident_bf[:])
            identf_sb = cp.tile([P, P], F32)
            nc.sync.dma_start(out=identf_sb[:], in_=ident_f32[:])

            xTs_sb = [cp.tile([P, NSH], BF16, name=f"xTs{k}") for k in range(KH)]
            for k in range(KH):
                nc.sync.dma_start(out=xTs_sb[k][:], in_=xTs[k * P:(k + 1) * P, :])
            h1T_sb = [cp.tile([P, NSH], BF16, name=f"h1T{k}") for k in range(KH)]

            # fused weights (resident, bf16)
            A1 = [[cp.tile([P, H], BF16, name=f"A1_{r}_{k}") for k in range(KH)] for r in range(R)]
            A1root = [cp.tile([P, H], BF16, name=f"A1root_{k}") for k in range(KH)]
            A2 = [[cp.tile([P, H], BF16, name=f"A2_{r}_{k}") for k in range(KH)] for r in range(R)]
            A2root = [cp.tile([P, H], BF16, name=f"A2root_{k}") for k in range(KH)]
            Caug1_sb = cp.tile([R + 1, H], F32)
            b2_sb = cp.tile([1, H], F32)
            nc.sync.dma_start(out=b2_sb[:], in_=b2row[:])

            # big DRAM intermediates
            caug_dram = dr.tile([R + 1, H], F32)
            msg1 = dr.tile([E1p, H], BF16)
            msg2 = dr.tile([E1p, H], BF16)
            h1f32 = dr.tile([NSH, H], F32)
            h1bf_sh = dr.tile([NSH, H], BF16)
            h1full = dr.tile([N, H], BF16, addr_space="Shared")
            poolin = dr.tile([G, H], F32)
            poolout = dr.tile([G, H], F32, addr_space="Shared")

            # ---------------- phase W: fused weight prep
            with (
                tc.tile_pool(name="wp", bufs=3) as wp,
                tc.tile_pool(name="wpp", bufs=2, space="PSUM") as wpp,
            ):
                WiT = [wp.tile([P, H], F32, name=f"WiT{k}", bufs=1) for k in range(KL)]
                for k in range(KL):
                    nc.sync.dma_start(out=WiT[k][:], in_=W_initT_d[k * P:(k + 1) * P, :])
                binit_sb = wp.tile([P, KL], F32, bufs=1)
                nc.sync.dma_start(out=binit_sb[:], in_=b_init_col[:])

                def fuse(dst_tiles, rhs_dram, bias_row_idx):
                    """dst[mc] = (W_init @ rhs)[mc*128:(mc+1)*128, :] as bf16;
                    also Caug1[bias_row_idx] = b_init @ rhs (f32)."""
                    rhs_t = [wp.tile([P, H], F32, name="fuserhs", tag="fuserhs", bufs=8)
                             for _ in range(KL)]
                    for k in range(KL):
                        nc.sync.dma_start(out=rhs_t[k][:], in_=rhs_dram[k * P:(k + 1) * P, :])
                    for mc in range(KH):
                        ps = wpp.tile([P, H], F32, name="fusepsum", tag="fps", bufs=4)
                        for k in range(KL):
                            nc.tensor.matmul(
                                out=ps[:], lhsT=WiT[k][:, mc * P:(mc + 1) * P],
                                rhs=rhs_t[k][:], start=(k == 0), stop=(k == KL - 1))
                        nc.vector.tensor_copy(out=dst_tiles[mc][:], in_=ps[:])
                    psb = wpp.tile([1, H], F32, name="fusebias", tag="fpb")
                    for k in range(KL):
                        nc.tensor.matmul(out=psb[:], lhsT=binit_sb[:, k:k + 1],
                                         rhs=rhs_t[k][:], start=(k == 0), stop=(k == KL - 1))
                    stage = wp.tile([1, H], F32, name="cstage", tag="cstage", bufs=2)
                    nc.vector.tensor_copy(out=stage[:], in_=psb[:])
                    nc.sync.dma_start(
                        out=caug_dram[bias_row_idx:bias_row_idx + 1, :], in_=stage[:])

                fuse(A1root, W_root1_d, 0)
                for r in range(R):
                    fuse(A1[r], W_rel1_d[r], 1 + r)
                nc.sync.dma_start(out=Caug1_sb[:], in_=caug_dram[:])
                # Caug1 row0 += b1
                b1_sb = wp.tile([1, H], F32)
                nc.sync.dma_start(out=b1_sb[:], in_=b1row[:])
                nc.vector.tensor_add(out=Caug1_sb[0:1, :], in0=Caug1_sb[0:1, :], in1=b1_sb[:])

                # layer-2 weights: straight bf16 casts
                for r in range(R):
                    for k in range(KH):
                        t = wp.tile([P, H], F32, name="w2ld", tag="w2ld", bufs=6)
                        nc.sync.dma_start(out=t[:], in_=W_rel2_d[r][k * P:(k + 1) * P, :])
                        nc.vector.tensor_copy(out=A2[r][k][:], in_=t[:])
                for k in range(KH):
                    t = wp.tile([P, H], F32, name="w2r", tag="w2ld", bufs=6)
                    nc.sync.dma_start(out=t[:], in_=W_root2_d[k * P:(k + 1) * P, :])
                    nc.vector.tensor_copy(out=A2root[k][:], in_=t[:])

            # ---------------- message pass (shared for both layers)
            def pass1(src_dram_bf, Ar, msgbuf):
                with (
                    tc.tile_pool(name="p1", bufs=6) as sp,
                    tc.tile_pool(name="p1p", bufs=4, space="PSUM") as pp,
                ):
                    t = 0
                    for r in range(R):
                        for _ in range(T1_r[r]):
                            g = sp.tile([P, H], BF16, name="g", tag="g")
                            nc.gpsimd.indirect_dma_start(
                                out=g[:], out_offset=None, in_=src_dram_bf[:],
                                in_offset=bass.IndirectOffsetOnAxis(
                                    ap=src1_sb[:, t:t + 1], axis=0))
                            gT = []
                            for k in range(KH):
                                tp = pp.tile([P, P], BF16, name="tp", tag="tp")
                                nc.tensor.transpose(
                                    out=tp[:], in_=g[:, k * P:(k + 1) * P],
                                    identity=identbf_sb[:])
                                gk = sp.tile([P, P], BF16, name="gT", tag=f"gT{k}")
                                nc.vector.tensor_copy(out=gk[:], in_=tp[:])
                                gT.append(gk)
                            mm = pp.tile([P, H], F32, name="mm", tag="mm")
                            for k in range(KH):
                                nc.tensor.matmul(out=mm[:], lhsT=gT[k][:], rhs=Ar[r][k][:],
                                                 start=(k == 0), stop=(k == KH - 1))
                            mb = sp.tile([P, H], BF16, name="mb", tag="mb")
                            nc.scalar.activation(out=mb[:], in_=mm[:], func=AF.Copy,
                                                 scale=w1_sb[:, t:t + 1])
                            nc.sync.dma_start(out=msgbuf[t * P:(t + 1) * P, :], in_=mb[:])
                            t += 1

            def pass2(msgbuf, Aroot, lhsT_sb, caug_k, on_block):
                with (
                    tc.tile_pool(name="p2", bufs=6) as sp,
                    tc.tile_pool(name="p2p", bufs=4, space="PSUM") as pp,
                ):
                    t = 0
                    for b in range(NB):
                        nb_sz = min(P, NSH - b * P)
                        acc = pp.tile([P, H], F32, name="acc", tag="acc")
                        for k in range(KH):
                            nc.tensor.matmul(
                                out=acc[:nb_sz], lhsT=lhsT_sb[k][:, b * P:b * P + nb_sz],
                                rhs=Aroot[k][:], start=(k == 0), stop=False)
                        nc.tensor.matmul(
                            out=acc[:nb_sz], lhsT=paug_sb[:caug_k, b * P:b * P + nb_sz],
                            rhs=(Caug1_sb[:caug_k, :] if caug_k > 1 else b2_sb[:]),
                            start=False, stop=(T2_b[b] == 0))
                        for _ in range(T2_b[b]):
                            mg = sp.tile([P, H], BF16, name="mg", tag="mg")
                            nc.gpsimd.indirect_dma_start(
                                out=mg[:], out_offset=None, in_=msgbuf[:],
                                in_offset=bass.IndirectOffsetOnAxis(
                                    ap=gidx2_sb[:, t:t + 1], axis=0))
                            oh = sp.tile([P, P], BF16, name="oh", tag="oh")
                            nc.vector.tensor_tensor(
                                out=oh[:], in0=slot2_sb[:, t:t + 1].to_broadcast([P, P]),
                                in1=iota128_sb[:], op=ALU.is_equal)
                            last = t == sum(T2_b[:b + 1]) - 1
                            nc.tensor.matmul(out=acc[:nb_sz], lhsT=oh[:, :nb_sz], rhs=mg[:],
                                             start=False, stop=last)
                            t += 1
                        on_block(b, nb_sz, acc, sp, pp)

            # ---------------- layer 1: host-pregathered feature-major stage;
            # 64-padded runs write msg rows directly at partition offsets.
            def pass1_direct(Ar, msgbuf):
                GB = 8
                rels = [r for r in range(R) for _ in range(T1_r[r])]
                with (
                    tc.tile_pool(name="p1d", bufs=4) as sp,
                    tc.tile_pool(name="p1dp", bufs=4, space="PSUM") as pp,
                ):
                    for g0 in range(0, T1, GB):
                        k_gr = min(GB, T1 - g0)
                        gw = [sp.tile([P, GB * P], BF16, name=f"gw{k}",
                                      tag=f"gw{k}") for k in range(KH)]
                        for k in range(KH):
                            nc.sync.dma_start(
                                out=gw[k][:, :k_gr * P],
                                in_=xg1T[k * P:(k + 1) * P, g0 * P:(g0 + k_gr) * P])
                        for j in range(k_gr):
                            t = g0 + j
                            mm = pp.tile([P, H], F32, name="mmd", tag="mmd")
                            for k in range(KH):
                                nc.tensor.matmul(
                                    out=mm[:], lhsT=gw[k][:, j * P:(j + 1) * P],
                                    rhs=Ar[rels[t]][k][:],
                                    start=(k == 0), stop=(k == KH - 1))
                            mb = sp.tile([P, H], BF16, name="mbd", tag="mbd")
                            nc.vector.tensor_scalar_mul(mb[:], mm[:], w1_sb[:, t:t + 1])
                            nc.sync.dma_start(out=msgbuf[t * P:(t + 1) * P, :],
                                              in_=mb[:])

            def l1_block(b, nb_sz, acc, sp, pp):
                h1b = sp.tile([P, H], F32, name="h1b", tag="h1b")
                nc.scalar.activation(out=h1b[:nb_sz], in_=acc[:nb_sz], func=AF.Relu)
                hbf = sp.tile([P, H], BF16, name="hbf", tag="hbf")
                nc.vector.tensor_copy(out=hbf[:nb_sz], in_=h1b[:nb_sz])
                nc.sync.dma_start(out=h1f32[b * P:b * P + nb_sz, :], in_=h1b[:nb_sz])
                nc.sync.dma_start(out=h1bf_sh[b * P:b * P + nb_sz, :], in_=hbf[:nb_sz])
                for k in range(KH):
                    tp = pp.tile([P, P], BF16, name="tph", tag="tph")
                    nc.tensor.transpose(out=tp[:, :nb_sz], in_=hbf[:nb_sz, k * P:(k + 1) * P],
                                        identity=identbf_sb[:nb_sz, :nb_sz])
                    nc.vector.tensor_copy(out=h1T_sb[k][:, b * P:b * P + nb_sz],
                                          in_=tp[:, :nb_sz])

            # ---------------- early head (pooling-independent attention branch)
            rcT = [cp.tile([P, G], F32, name=f"rcT{m}") for m in range(KL)]
            with (
                tc.tile_pool(name="eh", bufs=2) as hp,
                tc.tile_pool(name="ehp", bufs=2, space="PSUM") as hpp,
            ):
                def hps(p_, f_):
                    return hpp.tile([p_, f_], F32, name="ehps", tag="ehps")

                # rel8 [R, L] = rel_table @ W_init + b_init
                rtT = [hp.tile([P, R], F32, name=f"rtT{k}", bufs=1) for k in range(KH)]
                for k in range(KH):
                    nc.sync.dma_start(out=rtT[k][:], in_=rel_tableT[k * P:(k + 1) * P, :])
                Wi_sb = [hp.tile([P, L], F32, name=f"Wi{k}", bufs=1) for k in range(KH)]
                for k in range(KH):
                    nc.sync.dma_start(out=Wi_sb[k][:], in_=W_init_d[k * P:(k + 1) * P, :])
                rel8 = hp.tile([R, L], F32, bufs=1)
                for half in range(2):
                    sl = slice(half * (L // 2), (half + 1) * (L // 2))
                    ps = hps(R, L // 2)
                    for k in range(KH):
                        nc.tensor.matmul(out=ps[:], lhsT=rtT[k][:],
                                         rhs=Wi_sb[k][:, sl], start=(k == 0), stop=(k == KH - 1))
                    nc.vector.tensor_copy(out=rel8[:, sl], in_=ps[:])
                bi8 = hp.tile([R, L], F32, bufs=1)
                nc.sync.dma_start(out=bi8[:], in_=b_init_rep8[:])
                nc.vector.tensor_add(out=rel8[:], in0=rel8[:], in1=bi8[:])
                # rel8^T [L, R]
                r8T = [hp.tile([P, R], F32, name=f"r8T{k}", bufs=1) for k in range(KL)]
                for k in range(KL):
                    tp = hps(P, R)
                    nc.tensor.transpose(out=tp[:, :R], in_=rel8[:, k * P:(k + 1) * P],
                                        identity=identf_sb[:R, :R])
                    nc.vector.tensor_copy(out=r8T[k][:], in_=tp[:, :R])
                # t1T = tanh(Wa1^T @ rel8^T + ba1) [L, R]
                Wa1_sb = [hp.tile([P, L], F32, name=f"Wa1{k}", bufs=1) for k in range(KL)]
                for k in range(KL):
                    nc.sync.dma_start(out=Wa1_sb[k][:], in_=Wa1_d[k * P:(k + 1) * P, :])
                ba1_sb = hp.tile([P, KL], F32, bufs=1)
                nc.sync.dma_start(out=ba1_sb[:], in_=ba1_col[:])
                t1T = [hp.tile([P, R], F32, name=f"t1T{m}", bufs=1) for m in range(KL)]
                for m in range(KL):
                    ps = hps(P, R)
                    for k in range(KL):
                        nc.tensor.matmul(out=ps[:], lhsT=Wa1_sb[k][:, m * P:(m + 1) * P],
                                         rhs=r8T[k][:], start=(k == 0), stop=(k == KL - 1))
                    nc.scalar.activation(out=t1T[m][:], in_=ps[:], func=AF.Tanh,
                                         bias=ba1_sb[:, m:m + 1])
                # s8 [1, R]
                Wa2_sb = hp.tile([P, KL], F32, bufs=1)
                nc.sync.dma_start(out=Wa2_sb[:], in_=Wa2_col[:])
                ps8 = hps(1, R)
                for k in range(KL):
                    nc.tensor.matmul(out=ps8[:], lhsT=Wa2_sb[:, k:k + 1],
                                     rhs=t1T[k][:], start=(k == 0), stop=(k == KL - 1))
                s8 = hp.tile([1, R], F32, bufs=1)
                nc.scalar.add(out=s8[:], in_=ps8[:], add=float(ba2_val))
                # global softmax with counts
                smax = hp.tile([1, 1], F32, bufs=1)
                nc.vector.tensor_reduce(out=smax[:], in_=s8[:], axis=mybir.AxisListType.X,
                                        op=ALU.max)
                nsmax = hp.tile([1, 1], F32, bufs=1)
                nc.vector.tensor_scalar_mul(nsmax[:], smax[:], -1.0)
                e8 = hp.tile([1, R], F32, bufs=1)
                nc.scalar.activation(out=e8[:], in_=s8[:], func=AF.Exp, bias=nsmax[:, :1])
                nv_sb = hp.tile([1, R], F32, bufs=1)
                nc.sync.dma_start(out=nv_sb[:], in_=nvec[:])
                wsum = hp.tile([1, R], F32, bufs=1)
                nc.vector.tensor_mul(out=wsum[:], in0=e8[:], in1=nv_sb[:])
                den = hp.tile([1, 1], F32, bufs=1)
                nc.vector.tensor_reduce(out=den[:], in_=wsum[:], axis=mybir.AxisListType.X,
                                        op=ALU.add)
                rden = hp.tile([1, 1], F32, bufs=1)
                nc.vector.reciprocal(out=rden[:], in_=den[:])
                aval = hp.tile([1, R], F32, bufs=1)
                nc.vector.tensor_scalar_mul(aval[:], e8[:], rden[:, :1])
                # replicate aval over 64 partitions
                ones64_sb = hp.tile([1, G], F32, bufs=1)
                nc.sync.dma_start(out=ones64_sb[:], in_=ones1x64[:])
                avps = hps(G, R)
                nc.tensor.matmul(out=avps[:], lhsT=ones64_sb[:], rhs=aval[:],
                                 start=True, stop=True)
                avrep = hp.tile([G, R], F32, bufs=1)
                nc.vector.tensor_copy(out=avrep[:], in_=avps[:])
                # segment softmax via counts
                cg_sb = hp.tile([G, R], F32, bufs=1)
                nc.sync.dma_start(out=cg_sb[:], in_=cnt_gr[:])
                pn_sb = hp.tile([G, R], F32, bufs=1)
                nc.sync.dma_start(out=pn_sb[:], in_=presneg[:])
                gmin = hp.tile([G, R], F32, bufs=1)
                nc.vector.tensor_add(out=gmin[:], in0=avrep[:], in1=pn_sb[:])
                gmax = hp.tile([G, 1], F32, bufs=1)
                nc.vector.tensor_reduce(out=gmax[:], in_=gmin[:], axis=mybir.AxisListType.X,
                                        op=ALU.max)
                ngmax = hp.tile([G, 1], F32, bufs=1)
                nc.vector.tensor_scalar_mul(ngmax[:], gmax[:], -1.0)
                eg = hp.tile([G, R], F32, bufs=1)
                nc.scalar.activation(out=eg[:], in_=avrep[:], func=AF.Exp, bias=ngmax[:, :1])
                wden = hp.tile([G, R], F32, bufs=1)
                nc.vector.tensor_mul(out=wden[:], in0=eg[:], in1=cg_sb[:])
                deng = hp.tile([G, 1], F32, bufs=1)
                nc.vector.tensor_reduce(out=deng[:], in_=wden[:], axis=mybir.AxisListType.X,
                                        op=ALU.add)
                dengc = hp.tile([G, 1], F32, bufs=1)
                nc.vector.tensor_scalar_max(dengc[:], deng[:], 1e-30)
                rdeng = hp.tile([G, 1], F32, bufs=1)
                nc.vector.reciprocal(out=rdeng[:], in_=dengc[:])
                coef = hp.tile([G, R], F32, bufs=1)
                nc.vector.tensor_scalar_mul(coef[:], wden[:], rdeng[:, :1])
                # coef^T [R, G]
                cps = hps(R, G)
                nc.tensor.transpose(out=cps[:R, :], in_=coef[:, :R],
                                    identity=identf_sb[:G, :G])
                coefT = hp.tile([R, G], F32, bufs=1)
                nc.vector.tensor_copy(out=coefT[:], in_=cps[:R, :])
                # rel_ctx^T [L, G] = rel8^T @ coef^T
                for m in range(KL):
                    ps = hps(P, G)
                    nc.tensor.matmul(out=ps[:], lhsT=rel8[:, m * P:(m + 1) * P],
                                     rhs=coefT[:], start=True, stop=True)
                    nc.vector.tensor_copy(out=rcT[m][:], in_=ps[:])

            pass1_direct(A1, msg1)
            pass2(msg1, A1root, xTs_sb, R + 1, l1_block)

            # ---------------- AllGather h1 (bf16)
            nc.gpsimd.collective_compute(
                "AllGather", ALU.bypass,
                ins=[h1bf_sh.opt()], outs=[h1full.opt()],
                replica_groups=[list(range(C))])

            # ---------------- layer 2 (+ residual + pooling partials)
            poolsum_sb = cp.tile([G, H], F32)
            nc.vector.memset(poolsum_sb[:], 0.0)

            def l2_block(b, nb_sz, acc, sp, pp):
                t = sp.tile([P, H], F32, name="t2", tag="t2")
                nc.scalar.activation(out=t[:nb_sz], in_=acc[:nb_sz], func=AF.Relu)
                h1l = sp.tile([P, H], F32, name="h1l", tag="h1l")
                nc.sync.dma_start(out=h1l[:nb_sz], in_=h1f32[b * P:b * P + nb_sz, :])
                h2 = sp.tile([P, H], F32, name="h2", tag="h2")
                if nb_sz < P:
                    nc.vector.memset(h2[:], 0.0)
                nc.vector.tensor_add(out=h2[:nb_sz], in0=t[:nb_sz], in1=h1l[:nb_sz])
                nc.sync.dma_start(out=out_nodes[b * P:b * P + nb_sz, :], in_=h2[:nb_sz])
                oh64 = sp.tile([P, G], F32, name="oh64", tag="oh64")
                nc.vector.tensor_tensor(
                    out=oh64[:], in0=bslot_sb[:, b:b + 1].to_broadcast([P, G]),
                    in1=iota64_sb[:], op=ALU.is_equal)
                pps = pp.tile([G, H], F32, name="pps", tag="pps")
                nc.tensor.matmul(out=pps[:], lhsT=oh64[:], rhs=h2[:], start=True, stop=True)
                nc.vector.tensor_add(out=poolsum_sb[:], in0=poolsum_sb[:], in1=pps[:])

            pass1(h1full, A2, msg2)
            pass2(msg2, A2root, h1T_sb, 1, l2_block)

            # ---------------- AllReduce pooled sums
            nc.sync.dma_start(out=poolin[:], in_=poolsum_sb[:])
            nc.gpsimd.collective_compute(
                "AllReduce", ALU.add,
                ins=[poolin.opt()], outs=[poolout.opt()],
                replica_groups=[list(range(C))])

            # ---------------- head
            with (
                tc.tile_pool(name="hd", bufs=2) as hp,
                tc.tile_pool(name="hdp", bufs=2, space="PSUM") as hpp,
            ):
                K2L = 2 * L // P  # 12

                def hps(p_, f_):
                    return hpp.tile([p_, f_], F32, name="hps", tag="hps")

                # pooled mean [G, H]
                sums = hp.tile([G, H], F32, bufs=1)
                nc.sync.dma_start(out=sums[:], in_=poolout[:])
                cnts_sb = hp.tile([G, 1], F32, bufs=1)
                nc.sync.dma_start(out=cnts_sb[:], in_=cnts_nodes[:])
                rc = hp.tile([G, 1], F32, bufs=1)
                nc.vector.reciprocal(out=rc[:], in_=cnts_sb[:])
                mp = hp.tile([G, H], F32, bufs=1)
                nc.vector.tensor_scalar_mul(mp[:], sums[:], rc[:, :1])
                # meanpool^T [H, G]
                mpT = [hp.tile([P, G], F32, name=f"mpT{k}", bufs=1) for k in range(KH)]
                for k in range(KH):
                    tp = hps(P, G)
                    nc.tensor.transpose(out=tp[:], in_=mp[:, k * P:(k + 1) * P],
                                        identity=identf_sb[:G, :G])
                    nc.vector.tensor_copy(out=mpT[k][:], in_=tp[:])
                # graph_emb^T [L, G] = Wg^T @ mp^T + bg
                Wg_sb = [hp.tile([P, L], F32, name=f"Wg{k}", bufs=1) for k in range(KH)]
                for k in range(KH):
                    nc.sync.dma_start(out=Wg_sb[k][:], in_=Wg_d[k * P:(k + 1) * P, :])
                bg_sb = hp.tile([P, KL], F32, bufs=1)
                nc.sync.dma_start(out=bg_sb[:], in_=bg_col[:])
                geT = [hp.tile([P, G], F32, name=f"geT{m}", bufs=1) for m in range(KL)]
                for m in range(KL):
                    ps = hps(P, G)
                    for k in range(KH):
                        nc.tensor.matmul(out=ps[:], lhsT=Wg_sb[k][:, m * P:(m + 1) * P],
                                         rhs=mpT[k][:], start=(k == 0), stop=(k == KH - 1))
                    nc.vector.tensor_scalar_add(geT[m][:], ps[:], bg_sb[:, m:m + 1])

                # comb^T [OUT, G] = Wo^T @ [geT; rcT] + bo
                Wo_sb = [hp.tile([P, OUT], F32, name=f"Wo{k}", bufs=1) for k in range(K2L)]
                for k in range(K2L):
                    nc.sync.dma_start(out=Wo_sb[k][:], in_=Wo_d[k * P:(k + 1) * P, :])
                bo_sb = hp.tile([P, KO], F32, bufs=1)
                nc.sync.dma_start(out=bo_sb[:], in_=bo_col[:])
                cat = geT + rcT
                combT = [hp.tile([P, G], F32, name=f"combT{m}", bufs=1) for m in range(KO)]
                for m in range(KO):
                    ps = hps(P, G)
                    for k in range(K2L):
                        nc.tensor.matmul(out=ps[:], lhsT=Wo_sb[k][:, m * P:(m + 1) * P],
                                         rhs=cat[k][:], start=(k == 0), stop=(k == K2L - 1))
                    nc.vector.tensor_scalar_add(combT[m][:], ps[:], bo_sb[:, m:m + 1])
                # transpose back to [G, OUT]
                comb = hp.tile([G, OUT], F32, bufs=1)
                for m in range(KO):
                    tp = hps(G, P)
                    nc.tensor.transpose(out=tp[:G, :], in_=combT[m][:, :G],
                                        identity=identf_sb[:])
                    nc.vector.tensor_copy(out=comb[:, m * P:(m + 1) * P], in_=tp[:G, :])
                # layernorm over OUT
                nmu = hp.tile([G, 1], F32, bufs=1)
                nc.vector.tensor_reduce(out=nmu[:], in_=comb[:], axis=mybir.AxisListType.X,
                                        op=ALU.add)
                nc.vector.tensor_scalar_mul(nmu[:], nmu[:], -1.0 / OUT)
                xc = hp.tile([G, OUT], F32, bufs=1)
                nc.vector.tensor_scalar_add(xc[:], comb[:], nmu[:, :1])
                sq = hp.tile([G, OUT], F32, bufs=1)
                vsum = hp.tile([G, 1], F32, bufs=1)
                nc.scalar.activation(out=sq[:], in_=xc[:], func=AF.Square, accum_out=vsum[:])
                var = hp.tile([G, 1], F32, bufs=1)
                nc.vector.tensor_scalar(out=var[:], in0=vsum[:], scalar1=1.0 / OUT,
                                        scalar2=1e-5, op0=ALU.mult, op1=ALU.add)
                sd = hp.tile([G, 1], F32, bufs=1)
                nc.scalar.activation(out=sd[:], in_=var[:], func=AF.Sqrt)
                rsd = hp.tile([G, 1], F32, bufs=1)
                nc.vector.reciprocal(out=rsd[:], in_=sd[:])
                y = hp.tile([G, OUT], F32, bufs=1)
                nc.vector.tensor_scalar_mul(y[:], xc[:], rsd[:, :1])
                lg = hp.tile([G, OUT], F32, bufs=1)
                nc.sync.dma_start(out=lg[:], in_=lng_rep[:])
                lb = hp.tile([G, OUT], F32, bufs=1)
                nc.sync.dma_start(out=lb[:], in_=lnb_rep[:])
                nc.vector.tensor_mul(out=y[:], in0=y[:], in1=lg[:])
                nc.vector.tensor_add(out=y[:], in0=y[:], in1=lb[:])
                nc.sync.dma_start(out=out_graph[:], in_=y[:])

    nc.compile()
    return nc


# ---------------------------------------------------------------- runner

_CACHE = {}


def kernel(x, edge_index, edge_attr, batch, ptr, W_init, b_init, rel_table,
           W_root1, W_rel1, b1, W_root2, W_rel2, b2, Wg, bg, Wa1, ba1, Wa2, ba2,
           Wo, bo, ln_g, ln_b, _run_kwargs=None):
    x = np.asarray(x, dtype=np.float32)
    sched, arr = host_prep(x, np.asarray(edge_index), np.asarray(edge_attr),
                           np.asarray(ptr))

    key = (tuple(sched["T1_r"]), tuple(sched["T2_b"]),
           float(np.asarray(ba2).ravel()[0]))
    if key not in _CACHE:
        _CACHE.clear()
        _CACHE[key] = build(sched, float(np.asarray(ba2).ravel()[0]))
    nc = _CACHE[key]

    f32c = lambda a: np.ascontiguousarray(np.asarray(a, dtype=np.float32))
    iota128 = np.tile(np.arange(P, dtype=np.float32), (P, 1))
    iota64 = np.tile(np.arange(G, dtype=np.float32), (P, 1))
    ident = np.eye(P, dtype=np.float32)

    common = dict(
        iota128=iota128, iota64=iota64,
        ident_bf=ident.astype(bf16), ident_f32=ident,
        ones1x64=np.ones((1, G), np.float32),
        cnt_gr=arr["cnt_gr"], presneg=arr["presneg"], nvec=arr["nvec"],
        cnts_nodes=arr["cnts_nodes"],
        W_init=f32c(W_init), W_initT=f32c(np.asarray(W_init).T),
        b_init_col=np.ascontiguousarray(f32c(b_init).reshape(L // P, P).T),
        b_init_rep8=np.tile(f32c(b_init).reshape(1, L), (R, 1)),
        W_root1=f32c(W_root1), b1row=f32c(b1).reshape(1, H),
        W_rel1=f32c(W_rel1), W_root2=f32c(W_root2), b2row=f32c(b2).reshape(1, H),
        W_rel2=f32c(W_rel2), Wg=f32c(Wg),
        bg_col=np.ascontiguousarray(f32c(bg).reshape(L // P, P).T),
        Wa1=f32c(Wa1),
        ba1_col=np.ascontiguousarray(f32c(ba1).reshape(L // P, P).T),
        Wa2_col=np.ascontiguousarray(f32c(Wa2).reshape(L // P, P).T),
        rel_tableT=f32c(np.asarray(rel_table).T),
        Wo=f32c(Wo),
        bo_col=np.ascontiguousarray(f32c(bo).reshape(OUT // P, P).T),
        lng_rep=np.tile(f32c(ln_g).reshape(1, OUT), (G, 1)),
        lnb_rep=np.tile(f32c(ln_b).reshape(1, OUT), (G, 1)),
    )
    in_maps = []
    node_core, node_loc = arr["node_core"], arr["node_loc"]
    for c in range(C):
        m = dict(common)
        m["xg1T"] = np.ascontiguousarray(x[arr["src1_raw"][c]].T).astype(bf16)
        sel = node_core == c
        xc = np.zeros((NSH, H), dtype=np.float32)
        xc[node_loc[sel]] = x[sel]
        m["xTs"] = np.ascontiguousarray(xc.T).astype(bf16)
        m["src1"] = arr["src1"][c]
        m["w1"] = arr["w1"][c]
        m["gidx2"] = arr["gidx2"][c]
        m["slot2"] = arr["slot2"][c]
        m["paug"] = arr["paug"][c]
        m["batchslot"] = arr["batchslot"][c]
        in_maps.append(m)

    kw = _run_kwargs or {}
    res = run_bass_kernel_spmd(nc, in_maps, core_ids=list(range(C)), **kw)
    kernel._last_result = res
    node_emb = np.empty((N, H), dtype=np.float32)
    for c in range(C):
        sel = node_core == c
        node_emb[sel] = res.results[c]["out_nodes"][node_loc[sel]]
    out_g = res.results[0]["out_graph"]
    return node_emb, out_g
